# revision 1
# baseline (speedup 1.0000x reference)
"""GAT (2-layer graph attention network + output MLP) on 8 Trainium2 NeuronCores.

Strategy ("diagonal scheduling"):
  - The Bass program is built per-invocation, so the graph structure is a
    compile-time constant.  Nodes are assigned to cores balancing total
    in-degree, and within each core nodes are grouped into 128-node blocks
    sorted by (deg_lo, deg_hi) so that all nodes in a block have nearly equal
    in-degree from each half of the node space.
  - Edges of a block are laid out in "chunks" of 128 slots: slot (c, p) holds
    the c-th in-edge of the node on partition p.  A chunk therefore has at
    most one edge per destination, which turns the segment-softmax scatter
    into a plain PSUM accumulation with a constant identity stationary matrix
    (no masks, no segment ops).
  - Per layer, each core computes the feature/attention table rows for its own
    nodes ([h | alpha_src] per node), all-gathers the full table to DRAM, and
    then gathers per-edge rows with dma_gather (int16 indices force a lo/hi
    table split at NTOT/2).
  - softmax uses exp(leakyrelu(e)) = max(exp(e), exp(0.2 e)) and folds the
    1/z normalization after aggregation (exact same math as the reference,
    max-subtraction is skipped since logits are O(1)).

kernel(**inputs) -> np.ndarray  takes full inputs, returns the full output.
"""

import numpy as np

# ---------------------------------------------------------------- constants
N, E, F_IN, D_HID, H, N_CLS = 50000, 800000, 128, 96, 8, 40
DH = D_HID // H  # 12
NEG_SLOPE = 0.2
CORES = 8
BLK = 128
PAD_AS = -10000.0  # alpha_src for pad rows: exp(0.2*(PAD_AS+ad)) == 0.0
GK_MAX = 64  # max chunks per merged-gather group

_CACHE = {}


# ---------------------------------------------------------------- planning
def make_plan(edge_src, edge_dst, n=N, cores=CORES, blk=BLK):
    """Pure graph-structure planning (numpy only).

    Returns a dict with the node permutation, per-block common chunk counts
    and the per-core wrapped int16 gather-index arrays.
    """
    edge_src = np.asarray(edge_src, dtype=np.int64)
    edge_dst = np.asarray(edge_dst, dtype=np.int64)
    e = len(edge_src)

    deg = np.bincount(edge_dst, minlength=n)

    # nodes per core, including dummies; one dummy pinned last on every core
    npc = -(-(n + cores) // (cores * blk)) * blk  # round up to block multiple
    ntot = cores * npc
    half = ntot // 2

    # --- assign real nodes to cores balancing total degree (snake deal) ---
    order = np.argsort(-deg, kind="stable")  # real nodes by degree desc
    core_of = np.empty(n, dtype=np.int64)
    # snake pattern over rounds of 2*cores
    r = np.arange(n)
    rnd = r // cores
    pos = r % cores
    fwd = (rnd % 2) == 0
    lane = np.where(fwd, pos, cores - 1 - pos)
    core_of[order] = lane

    # lo set = cores 0..cores/2-1
    is_lo_node = core_of < (cores // 2)
    src_is_lo = is_lo_node[edge_src]
    d_lo = np.bincount(edge_dst[src_is_lo], minlength=n)
    d_hi = deg - d_lo

    # --- per-half global ordering, dealt round-robin to the half's cores ---
    # Sorting each half globally by (d_lo desc, d_hi snake) and dealing node
    # at sorted position g to core g%hc, slot g//hc keeps every core's block
    # profile an interleaved sample of the same distribution, so the common
    # (cross-core max) chunk counts stay tight.  Dummies sort last, which
    # pins one dummy at the final slot of every core (used as the pad row).
    new_of_old = np.empty(n, dtype=np.int64)
    old_of_new = np.full(ntot, -1, dtype=np.int64)
    hc = cores // 2
    for side in (0, 1):
        mine = np.where(is_lo_node == (side == 0))[0]
        dl, dhh = d_lo[mine], d_hi[mine]
        run_parity = (dl.max() - dl) % 2  # alternate d_hi dir per d_lo run
        key_hi = np.where(run_parity == 0, -dhh, dhh)
        srt = mine[np.lexsort((key_hi, -dl))]
        assert len(srt) <= hc * npc - hc, (len(srt), npc)
        g = np.arange(len(srt))
        core = side * hc + g % hc
        slot = g // hc
        newids = core * npc + slot
        new_of_old[srt] = newids
        old_of_new[newids] = srt

    nblk = npc // blk  # blocks per core

    # --- per (core, block) lo/hi chunk counts -> common across cores ---
    d_lo_new = np.zeros(ntot, dtype=np.int64)
    d_hi_new = np.zeros(ntot, dtype=np.int64)
    real = old_of_new >= 0
    d_lo_new[real] = d_lo[old_of_new[real]]
    d_hi_new[real] = d_hi[old_of_new[real]]
    # block max per core
    kl_cb = d_lo_new.reshape(cores, nblk, blk).max(axis=2)
    kh_cb = d_hi_new.reshape(cores, nblk, blk).max(axis=2)
    K_lo = kl_cb.max(axis=0)  # [nblk] common
    K_hi = kh_cb.max(axis=0)

    off_lo = np.concatenate([[0], np.cumsum(K_lo * blk)])  # slot offsets
    off_hi = np.concatenate([[0], np.cumsum(K_hi * blk)])
    S_lo = int(off_lo[-1])
    S_hi = int(off_hi[-1])

    pad_lo = half - 1  # last node of core cores/2-1 (pinned dummy)
    pad_hi = half - 1  # (value in hi-table local coords: ntot-1-half)

    # --- slot filling ---
    dst_new = new_of_old[edge_dst]
    src_new = new_of_old[edge_src]
    is_lo = src_new < half

    # rank of each edge within its (dst, class) group
    # sort edges by (class, dst_new) then rank = position - group start
    grp = dst_new * 2 + (~is_lo)  # group id
    srt = np.argsort(grp, kind="stable")
    grp_s = grp[srt]
    starts = np.concatenate([[0], np.where(np.diff(grp_s) != 0)[0] + 1])
    group_start = np.zeros(len(grp_s), dtype=np.int64)
    group_start[starts] = starts
    group_start = np.maximum.accumulate(group_start)
    rank_s = np.arange(e) - group_start
    rank = np.empty(e, dtype=np.int64)
    rank[srt] = rank_s

    core_e = dst_new // npc
    blk_e = (dst_new % npc) // blk
    p_e = dst_new % blk

    slots_lo = np.full((cores, S_lo), pad_lo, dtype=np.int16)
    slots_hi = np.full((cores, S_hi), pad_hi, dtype=np.int16)

    lo_m = is_lo
    pos_lo = off_lo[blk_e[lo_m]] + rank[lo_m] * blk + p_e[lo_m]
    slots_lo[core_e[lo_m], pos_lo] = src_new[lo_m].astype(np.int16)
    hi_m = ~is_lo
    pos_hi = off_hi[blk_e[hi_m]] + rank[hi_m] * blk + p_e[hi_m]
    slots_hi[core_e[hi_m], pos_hi] = (src_new[hi_m] - half).astype(np.int16)

    # wrap for dma_gather: element i -> [i%16, i//16], tiled to 128 partitions
    def wrap(a):
        # a: [cores, S] -> [cores, 128, S//16]
        s = a.shape[1]
        if s == 0:
            return np.zeros((cores, 128, 0), dtype=np.int16)
        w = a.reshape(cores, s // 16, 16).transpose(0, 2, 1)  # [cores,16,S/16]
        return np.ascontiguousarray(np.tile(w, (1, 8, 1)))

    # --- group consecutive blocks for merged gathers ---
    groups = []  # list of (b0, b1)  [b0, b1) blocks
    b0 = 0
    ktot = K_lo + K_hi
    while b0 < nblk:
        b1 = b0 + 1
        s = int(ktot[b0])
        while b1 < nblk and s + int(ktot[b1]) <= GK_MAX:
            s += int(ktot[b1])
            b1 += 1
        groups.append((b0, b1))
        b0 = b1

    return dict(
        n=n, e=e, cores=cores, npc=npc, ntot=ntot, half=half, nblk=nblk,
        new_of_old=new_of_old, old_of_new=old_of_new,
        K_lo=K_lo.astype(np.int64), K_hi=K_hi.astype(np.int64),
        off_lo=off_lo, off_hi=off_hi, S_lo=S_lo, S_hi=S_hi,
        idx_lo=wrap(slots_lo), idx_hi=wrap(slots_hi),
        groups=groups,
        util=float(e) / max(1.0, float((S_lo + S_hi) * cores)),
    )


# ---------------------------------------------------------------- program
def build_program(plan, f_in=F_IN, d_hid=D_HID, h=H, n_cls=N_CLS, bf16=False,
                  stop_after=None, repeat=1, mock_cc=False, acc="pe"):
    import concourse.bacc as bacc
    import concourse.mybir as mybir
    from concourse import tile

    dt = mybir.dt
    f32 = dt.float32
    TDT = dt.bfloat16 if bf16 else dt.float32
    dh = d_hid // h
    npc, nblk, half = plan["npc"], plan["nblk"], plan["half"]
    K_lo, K_hi = plan["K_lo"], plan["K_hi"]
    off_lo, off_hi = plan["off_lo"], plan["off_hi"]
    S_lo, S_hi = plan["S_lo"], plan["S_hi"]
    cores = plan["cores"]
    ntot = plan["ntot"]
    ROW = 128  # table row, elements (512B fp32 / 256B bf16)
    DCAT = d_hid + 2 * h  # 112

    nc = bacc.Bacc("TRN2", target_bir_lowering=False, debug=False,
                   num_devices=cores)

    # ---- I/O ----
    xT = nc.dram_tensor("xT", [f_in, npc], f32, kind="ExternalInput")
    W0cat = nc.dram_tensor("W0cat", [f_in, DCAT], f32, kind="ExternalInput")
    W1cat = nc.dram_tensor("W1cat", [d_hid, DCAT], f32, kind="ExternalInput")
    Wout = nc.dram_tensor("Wout", [d_hid, n_cls], f32, kind="ExternalInput")
    b0b = nc.dram_tensor("b0b", [128, d_hid], f32, kind="ExternalInput")
    b1b = nc.dram_tensor("b1b", [128, d_hid], f32, kind="ExternalInput")
    boutb = nc.dram_tensor("boutb", [128, n_cls], f32, kind="ExternalInput")
    identt = nc.dram_tensor("identt", [128, 128], TDT, kind="ExternalInput")
    ident32 = nc.dram_tensor("ident32", [128, 128], f32, kind="ExternalInput")
    idx_lo_d = nc.dram_tensor("idx_lo", [128, max(S_lo // 16, 1)], dt.int16,
                              kind="ExternalInput")
    idx_hi_d = nc.dram_tensor("idx_hi", [128, max(S_hi // 16, 1)], dt.int16,
                              kind="ExternalInput")
    out_d = nc.dram_tensor("out", [npc, n_cls], f32, kind="ExternalOutput")

    Kmax = int(max(1, (K_lo + K_hi).max()))

    with tile.TileContext(nc) as tc:
        with (
            tc.tile_pool(name="dram", bufs=1, space="DRAM") as dramp,
            tc.tile_pool(name="persist", bufs=1) as pers,
            tc.tile_pool(name="gath", bufs=3) as gath,
            tc.tile_pool(name="stage", bufs=3) as stage,
            tc.tile_pool(name="small", bufs=4) as small,
            tc.tile_pool(name="psA", bufs=2, space="PSUM") as psA,
            tc.tile_pool(name="psB", bufs=2, space="PSUM") as psB,
            tc.tile_pool(name="psT", bufs=2, space="PSUM") as psT,
        ):
            # ---- DRAM scratch ----
            tslice = dramp.tile([npc, ROW], TDT)
            aspace = "Local" if mock_cc else "Shared"
            tfulls = [
                (dramp.tile([ntot, ROW], TDT, addr_space=aspace,
                            name=f"tf0_{r}", tag=f"tf0_{r}"),
                 dramp.tile([ntot, ROW], TDT, addr_space=aspace,
                            name=f"tf1_{r}", tag=f"tf1_{r}"))
                for r in range(repeat)
            ]

            # ---- persistent SBUF ----
            xT_sb = pers.tile([f_in, npc], f32)
            nc.sync.dma_start(xT_sb[:], xT[:, :])
            W0_sb = pers.tile([f_in, DCAT], f32)
            nc.sync.dma_start(W0_sb[:], W0cat[:, :])
            W1_sb = pers.tile([d_hid, DCAT], f32)
            nc.sync.dma_start(W1_sb[:], W1cat[:, :])
            Wo_sb = pers.tile([d_hid, n_cls], f32)
            nc.sync.dma_start(Wo_sb[:], Wout[:, :])
            b0_sb = pers.tile([128, d_hid], f32)
            nc.sync.dma_start(b0_sb[:], b0b[:, :])
            b1_sb = pers.tile([128, d_hid], f32)
            nc.sync.dma_start(b1_sb[:], b1b[:, :])
            bo_sb = pers.tile([128, n_cls], f32)
            nc.sync.dma_start(bo_sb[:], boutb[:, :])
            idt_sb = pers.tile([128, 128], TDT)
            nc.sync.dma_start(idt_sb[:], identt[:, :])
            id32_sb = pers.tile([128, 128], f32)
            nc.sync.dma_start(id32_sb[:], ident32[:, :])
            if S_lo:
                ixlo_sb = pers.tile([128, S_lo // 16], dt.int16)
                nc.sync.dma_start(ixlo_sb[:], idx_lo_d[:, :])
            if S_hi:
                ixhi_sb = pers.tile([128, S_hi // 16], dt.int16)
                nc.sync.dma_start(ixhi_sb[:], idx_hi_d[:, :])
            ad0_sb = pers.tile([128, nblk * h], TDT)
            ad1_sb = pers.tile([128, nblk * h], TDT)
            h1_sb = pers.tile([128, nblk * d_hid], f32)
            h2_sb = pers.tile([128, nblk * d_hid], f32)
            padrow = pers.tile([1, h], TDT)
            nc.vector.memset(padrow[:], PAD_AS)

            # ================= helper: table build =================
            def table_build(src_lhsT, Wc_sb, ad_sb, tf):
                """src_lhsT(b) -> lhsT AP [k, 128] for block b."""
                for b in range(nblk):
                    lhsT = src_lhsT(b)
                    ps = psA.tile([128, DCAT], mybir.dt.float32)
                    nc.tensor.matmul(ps[:], lhsT, Wc_sb[:, :], start=True,
                                     stop=True)
                    stg = stage.tile([128, ROW], TDT, tag="stg")
                    nc.vector.tensor_copy(stg[:, 0:DCAT], ps[:, 0:DCAT])
                    nc.vector.memset(stg[:, DCAT:ROW], 0.0)
                    nc.vector.tensor_copy(
                        ad_sb[:, b * h:(b + 1) * h],
                        ps[:, d_hid + h:d_hid + 2 * h])
                    nc.sync.dma_start(
                        tslice[b * 128:(b + 1) * 128, :], stg[:])
                # pad row: overwrite alpha_src of the core's last node
                nc.sync.dma_start(
                    tslice[npc - 1:npc, d_hid:d_hid + h], padrow[:])
                if mock_cc:
                    # cost-model stand-in: move the same bytes the AllGather
                    # would receive (cores-1 slices in + 1 local copy)
                    for c in range(cores):
                        nc.sync.dma_start(
                            tf[c * npc:(c + 1) * npc, :], tslice[:, :])
                else:
                    nc.gpsimd.collective_compute(
                        "AllGather", mybir.AluOpType.bypass,
                        replica_groups=[list(range(cores))],
                        ins=[tslice[:, :]], outs=[tf[:, :]])

            # ================= helper: edge phase =================
            def edge_phase(tf, ad_sb, post):
                """post(b, ps) consumes psum [128, d_hid+h] for block b.

                Gathers are merged across groups of consecutive blocks to
                amortize the ~1us SWDGE fixed cost per dma_gather; edge math
                runs group-wide where it is block-agnostic.
                """
                for (g0, g1) in plan["groups"]:
                    KLg = int(off_lo[g1] - off_lo[g0]) // 128
                    KHg = int(off_hi[g1] - off_hi[g0]) // 128
                    Kg = KLg + KHg
                    if Kg == 0:
                        for b in range(g0, g1):
                            post(b, None)
                        continue
                    G = gath.tile([128, GK_MAX * 128], TDT, tag="G")
                    if KLg:
                        nc.gpsimd.dma_gather(
                            G[:, :KLg * 128].rearrange("p (k e) -> p k e",
                                                       e=128),
                            tf[0:half, :],
                            ixlo_sb[:, off_lo[g0] // 16:off_lo[g1] // 16],
                            128 * KLg, 128 * KLg, ROW,
                            single_packet=False)
                    if KHg:
                        nc.gpsimd.dma_gather(
                            G[:, KLg * 128:Kg * 128].rearrange(
                                "p (k e) -> p k e", e=128),
                            tf[half:ntot, :],
                            ixhi_sb[:, off_hi[g0] // 16:off_hi[g1] // 16],
                            128 * KHg, 128 * KHg, ROW,
                            single_packet=False)
                    Gv = G[:, :Kg * 128].rearrange("p (k e) -> p k e", e=128)

                    def blk_ranges(b):
                        lo = ((off_lo[b] - off_lo[g0]) // 128,
                              (off_lo[b + 1] - off_lo[g0]) // 128)
                        hi = (KLg + (off_hi[b] - off_hi[g0]) // 128,
                              KLg + (off_hi[b + 1] - off_hi[g0]) // 128)
                        return [r for r in (lo, hi) if r[1] > r[0]]

                    # e = alpha_src + alpha_dst (per block: ad varies)
                    for b in range(g0, g1):
                        adc = ad_sb[:, b * h:(b + 1) * h]
                        for (c0, c1) in blk_ranges(b):
                            kk = int(c1 - c0)
                            nc.vector.tensor_add(
                                Gv[:, c0:c1, 112:120],
                                Gv[:, c0:c1, d_hid:d_hid + h],
                                adc.rearrange("p (o j) -> p o j",
                                              o=1).broadcast_to([128, kk, h]))
                    # group-wide: s_exp = max(exp(e), exp(0.2 e)); msg scale
                    asv = Gv[:, :, d_hid:d_hid + h]
                    ev = Gv[:, :, 112:120]
                    uv = Gv[:, :, 120:128]
                    nc.scalar.activation(uv, ev,
                                         mybir.ActivationFunctionType.Exp)
                    nc.scalar.activation(asv, ev,
                                         mybir.ActivationFunctionType.Exp,
                                         scale=NEG_SLOPE)
                    nc.vector.tensor_max(asv, asv, uv)
                    msgv = Gv[:, :, 0:d_hid].rearrange(
                        "p k (j d) -> p k j d", d=dh)
                    sexp = asv.rearrange("p k (j o) -> p k j o",
                                         o=1).broadcast_to([128, Kg, h, dh])
                    nc.vector.tensor_mul(msgv, msgv, sexp)
                    # per-block accumulate [msg | s_exp] via identity matmul
                    for b in range(g0, g1):
                        chunks = [c for (c0, c1) in blk_ranges(b)
                                  for c in range(c0, c1)]
                        if not chunks:
                            post(b, None)
                            continue
                        ps = psB.tile([128, d_hid + h], mybir.dt.float32)
                        for i, c in enumerate(chunks):
                            nc.tensor.matmul(
                                ps[:], idt_sb[:, :],
                                G[:, c * 128:c * 128 + d_hid + h],
                                start=(i == 0), stop=(i == len(chunks) - 1))
                        post(b, ps)

            # ================= phase A: table 0 =================
            for _rep in range(repeat):
              tfull0, tfull1 = tfulls[_rep]
              table_build(
                lambda b: xT_sb[:, b * 128:(b + 1) * 128],
                W0_sb, ad0_sb, tfull0)

              def bail():
                  # drain something visible to out_d so the program stays valid
                  t = stage.tile([128, n_cls], mybir.dt.float32, tag="t0")
                  nc.vector.memset(t[:], 0.0)
                  for b in range(nblk):
                      nc.sync.dma_start(out_d[b * 128:(b + 1) * 128, :], t[:])

              if stop_after == "A":
                  bail()

              # ================= phase B: layer-0 edges =================
              def post0(b, ps):
                  hv = h1_sb[:, b * d_hid:(b + 1) * d_hid]
                  if ps is None:
                      nc.vector.tensor_copy(hv, b0_sb[:, :])
                      return
                  z = small.tile([128, h], mybir.dt.float32, tag="z")
                  nc.vector.tensor_scalar_add(z[:], ps[:, d_hid:d_hid + h],
                                              1e-16)
                  iz = small.tile([128, h], mybir.dt.float32, tag="iz")
                  nc.vector.reciprocal(iz[:], z[:])
                  izb = iz[:, :].rearrange("p (j o) -> p j o",
                                           o=1).broadcast_to([128, h, dh])
                  hv3 = hv.rearrange("p (j d) -> p j d", d=dh)
                  nc.vector.tensor_mul(hv3, ps[:, 0:d_hid].rearrange(
                      "p (j d) -> p j d", d=dh), izb)
                  nc.vector.tensor_add(hv, hv, b0_sb[:, :])

              if stop_after not in ("A",):
                  edge_phase(tfull0, ad0_sb, post0)
              if stop_after == "B":
                  bail()

              # ================= phase C: table 1 =================
              def lhsT1(b):
                  pst = psT.tile([d_hid, 128], mybir.dt.float32, tag="ptr")
                  nc.tensor.transpose(
                      pst[:], h1_sb[:, b * d_hid:(b + 1) * d_hid],
                      id32_sb[:, :])
                  hT = stage.tile([d_hid, 128], mybir.dt.float32, tag="hT")
                  nc.vector.tensor_copy(hT[:], pst[:])
                  return hT[:, :]

              if stop_after not in ("A", "B"):
                  table_build(lhsT1, W1_sb, ad1_sb, tfull1)
              if stop_after == "C":
                  bail()

              # ================= phase D: layer-1 edges =================
              def post1(b, ps):
                  hv = h2_sb[:, b * d_hid:(b + 1) * d_hid]
                  if ps is None:
                      t = small.tile([128, d_hid], mybir.dt.float32, tag="t1")
                      nc.vector.tensor_copy(t[:], b1_sb[:, :])
                      nc.vector.tensor_scalar_max(hv, t[:], 0.0)
                      return
                  z = small.tile([128, h], mybir.dt.float32, tag="z")
                  nc.vector.tensor_scalar_add(z[:], ps[:, d_hid:d_hid + h],
                                              1e-16)
                  iz = small.tile([128, h], mybir.dt.float32, tag="iz")
                  nc.vector.reciprocal(iz[:], z[:])
                  izb = iz[:, :].rearrange("p (j o) -> p j o",
                                           o=1).broadcast_to([128, h, dh])
                  t = small.tile([128, d_hid], mybir.dt.float32, tag="t1")
                  t3 = t[:, :].rearrange("p (j d) -> p j d", d=dh)
                  nc.vector.tensor_mul(t3, ps[:, 0:d_hid].rearrange(
                      "p (j d) -> p j d", d=dh), izb)
                  nc.vector.tensor_add(t[:], t[:], b1_sb[:, :])
                  nc.vector.tensor_scalar_max(hv, t[:], 0.0)  # ReLU

              if stop_after not in ("A", "B", "C"):
                  edge_phase(tfull1, ad1_sb, post1)
              if stop_after == "D":
                  bail()

              # ================= phase E: output MLP + log_softmax ========
              skipE = stop_after in ("A", "B", "C", "D")
              for b in range(nblk if not skipE else 0):
                  pst = psT.tile([d_hid, 128], mybir.dt.float32, tag="ptr")
                  nc.tensor.transpose(
                      pst[:], h2_sb[:, b * d_hid:(b + 1) * d_hid],
                      id32_sb[:, :])
                  hT = stage.tile([d_hid, 128], mybir.dt.float32, tag="hT")
                  nc.vector.tensor_copy(hT[:], pst[:])
                  po = psT.tile([128, n_cls], mybir.dt.float32, tag="po")
                  nc.tensor.matmul(po[:], hT[:, :], Wo_sb[:, :], start=True,
                                   stop=True)
                  t0 = stage.tile([128, n_cls], mybir.dt.float32, tag="t0")
                  nc.vector.tensor_add(t0[:], po[:], bo_sb[:, :])
                  m = small.tile([128, 1], mybir.dt.float32, tag="m")
                  nc.vector.reduce_max(m[:], t0[:],
                                       axis=mybir.AxisListType.X)
                  nc.vector.tensor_scalar(t0[:], t0[:], m[:, 0:1], None,
                                          op0=mybir.AluOpType.subtract)
                  ex = stage.tile([128, n_cls], mybir.dt.float32, tag="ex")
                  nc.scalar.activation(ex[:], t0[:],
                                       mybir.ActivationFunctionType.Exp)
                  s = small.tile([128, 1], mybir.dt.float32, tag="s")
                  nc.vector.reduce_sum(s[:], ex[:],
                                       axis=mybir.AxisListType.X)
                  ls = small.tile([128, 1], mybir.dt.float32, tag="ls")
                  nc.scalar.activation(ls[:], s[:],
                                       mybir.ActivationFunctionType.Ln)
                  nc.vector.tensor_scalar(t0[:], t0[:], ls[:, 0:1], None,
                                          op0=mybir.AluOpType.subtract)
                  nc.sync.dma_start(out_d[b * 128:(b + 1) * 128, :], t0[:])

    nc.compile()
    return nc


# ---------------------------------------------------------------- inputs
def make_in_maps(plan, inputs, f_in=F_IN, d_hid=D_HID, h=H, n_cls=N_CLS,
                 bf16=False):
    import ml_dtypes  # noqa: F401

    x = np.asarray(inputs["x"], dtype=np.float32)
    W0 = np.asarray(inputs["W0"], dtype=np.float32)
    W1 = np.asarray(inputs["W1"], dtype=np.float32)
    Wout = np.asarray(inputs["Wout"], dtype=np.float32)
    as0 = np.asarray(inputs["as0"], dtype=np.float32)
    ad0 = np.asarray(inputs["ad0"], dtype=np.float32)
    as1 = np.asarray(inputs["as1"], dtype=np.float32)
    ad1 = np.asarray(inputs["ad1"], dtype=np.float32)
    b0 = np.asarray(inputs["b0"], dtype=np.float32)
    b1 = np.asarray(inputs["b1"], dtype=np.float32)
    bout = np.asarray(inputs["bout"], dtype=np.float32)

    dh = d_hid // h
    npc, cores = plan["npc"], plan["cores"]
    old_of_new = plan["old_of_new"]

    def blockdiag(a):  # [h, dh] -> [d_hid, h]
        m = np.zeros((d_hid, h), dtype=np.float32)
        for j in range(h):
            m[j * dh:(j + 1) * dh, j] = a[j]
        return m

    W0cat = np.concatenate(
        [W0, W0 @ blockdiag(as0), W0 @ blockdiag(ad0)], axis=1)
    W1cat = np.concatenate(
        [W1, W1 @ blockdiag(as1), W1 @ blockdiag(ad1)], axis=1)

    tdt = ml_dtypes.bfloat16 if bf16 else np.float32
    ident = np.eye(128, dtype=tdt)
    ident32 = np.eye(128, dtype=np.float32)
    b0b = np.ascontiguousarray(np.broadcast_to(b0, (128, d_hid)))
    b1b = np.ascontiguousarray(np.broadcast_to(b1, (128, d_hid)))
    boutb = np.ascontiguousarray(np.broadcast_to(bout, (128, n_cls)))

    x_ext = np.zeros((npc * cores, f_in), dtype=np.float32)
    real = old_of_new >= 0
    x_ext[real] = x[old_of_new[real]]

    in_maps = []
    for c in range(cores):
        xs = x_ext[c * npc:(c + 1) * npc]
        m = dict(
            xT=np.ascontiguousarray(xs.T),
            W0cat=W0cat, W1cat=W1cat, Wout=Wout,
            b0b=b0b, b1b=b1b, boutb=boutb,
            identt=ident, ident32=ident32,
            idx_lo=np.ascontiguousarray(plan["idx_lo"][c])
            if plan["S_lo"] else np.zeros((128, 1), np.int16),
            idx_hi=np.ascontiguousarray(plan["idx_hi"][c])
            if plan["S_hi"] else np.zeros((128, 1), np.int16),
        )
        in_maps.append(m)
    return in_maps


def assemble_output(plan, results, n_cls=N_CLS):
    outs = np.concatenate([r["out"] for r in results], axis=0)
    return np.ascontiguousarray(outs[plan["new_of_old"]], dtype=np.float32)


# ---------------------------------------------------------------- entry
def kernel(**inputs):
    from concourse.bass_utils import run_bass_kernel_spmd

    edge_src = np.asarray(inputs["edge_src"]).astype(np.int64)
    edge_dst = np.asarray(inputs["edge_dst"]).astype(np.int64)

    bf16 = True
    key = (edge_src.tobytes(), edge_dst.tobytes(), bf16)
    kh = hash(key)
    if kh not in _CACHE:
        plan = make_plan(edge_src, edge_dst)
        nc = build_program(plan, bf16=bf16)
        _CACHE[kh] = (plan, nc)
    plan, nc = _CACHE[kh]

    in_maps = make_in_maps(plan, inputs, bf16=bf16)
    res = run_bass_kernel_spmd(nc, in_maps,
                               core_ids=list(range(plan["cores"])))
    return assemble_output(plan, res.results)



# revision 12
# speedup vs baseline: 1.4671x; 1.4671x over previous
"""GAT (2-layer graph attention network + output MLP) on 8 Trainium2 NeuronCores.

Strategy ("diagonal scheduling", v2):
  - The Bass program is built per-invocation, so the graph structure is a
    compile-time constant.  Nodes are assigned to cores balancing total
    in-degree, and within each core nodes are grouped into 128-node blocks
    sorted by (deg_lo, deg_hi) so that all nodes in a block have nearly equal
    in-degree from each half of the node space.
  - Edges of a block are laid out in "chunks" of 128 slots: slot (c, p) holds
    the c-th in-edge of the node on partition p.  A chunk therefore has at
    most one edge per destination, which turns the segment-softmax scatter
    into a plain PSUM accumulation with a constant identity stationary matrix
    (no masks, no segment ops).
  - Per layer, each core computes the feature/attention table rows for its own
    nodes ([h | alpha_src | alpha_dst] per node), all-gathers the full table to
    DRAM, and then gathers per-edge rows with dma_gather (int16 indices force
    a lo/hi table split at NTOT/2).
  - Table rows are numbered partition-major (row = p*nblk + b) so the whole
    per-core table slice is written to DRAM with ONE contiguous DMA straight
    from the SBUF-resident table tile (which also serves as the local
    alpha_dst source).  The PSUM->SBUF bf16 convert runs on the otherwise-idle
    Activation engine.
  - softmax: exp(leakyrelu(e)) == exp(max(e, 0.2e)) -- one DVE
    scalar_tensor_tensor (mult/max), then two ACT exps: an 8-wide one (the
    per-edge weights accumulated for the softmax denominator) and a 12-wide
    broadcast-EXPANDED one so the message multiply is a fully-packed bf16
    TensorTensor (2x DVE mode).  The 1/z normalization folds in after
    aggregation (exact same math as the reference; max-subtraction is skipped
    since logits are O(1)).
  - The output MLP + log_softmax is folded into layer-1's per-block epilogue
    and accumulated in SBUF; one contiguous DMA writes the result at the end.

kernel(**inputs) -> np.ndarray  takes full inputs, returns the full output.
"""

import numpy as np

# ---------------------------------------------------------------- constants
N, E, F_IN, D_HID, H, N_CLS = 50000, 800000, 128, 96, 8, 40
DH = D_HID // H  # 12
NEG_SLOPE = 0.2
CORES = 8
BLK = 128
PAD_AS = -10000.0  # alpha_src for pad rows: exp(leaky(PAD_AS+ad)) == 0.0
GK_MAX = 64  # max chunks per merged-gather group

_CACHE = {}


# ---------------------------------------------------------------- planning
def make_plan(edge_src, edge_dst, n=N, cores=CORES, blk=BLK):
    """Pure graph-structure planning (numpy only).

    Returns a dict with the node permutations, per-block common chunk counts
    and the per-core wrapped int16 gather-index arrays.  Node ids come in two
    numberings: "slot-major" (slot s = b*128 + p; used for the xT input
    layout) and "row-major" (table row r = p*nblk + b; used for the DRAM
    table, the gather indices and the output layout).
    """
    edge_src = np.asarray(edge_src, dtype=np.int64)
    edge_dst = np.asarray(edge_dst, dtype=np.int64)
    e = len(edge_src)

    deg = np.bincount(edge_dst, minlength=n)

    # nodes per core, including dummies; one dummy pinned last on every core
    npc = -(-(n + cores) // (cores * blk)) * blk  # round up to block multiple
    ntot = cores * npc
    half = ntot // 2
    nblk = npc // blk

    # --- assign real nodes to cores balancing total degree (snake deal) ---
    order = np.argsort(-deg, kind="stable")  # real nodes by degree desc
    core_of = np.empty(n, dtype=np.int64)
    r = np.arange(n)
    rnd = r // cores
    pos = r % cores
    fwd = (rnd % 2) == 0
    lane = np.where(fwd, pos, cores - 1 - pos)
    core_of[order] = lane

    # lo set = cores 0..cores/2-1
    is_lo_node = core_of < (cores // 2)
    src_is_lo = is_lo_node[edge_src]
    d_lo = np.bincount(edge_dst[src_is_lo], minlength=n)
    d_hi = deg - d_lo

    # --- per-half global ordering, dealt round-robin to the half's cores ---
    # Sorting each half globally by (d_lo desc, d_hi snake) and dealing node
    # at sorted position g to core g%hc, slot g//hc keeps every core's block
    # profile an interleaved sample of the same distribution, so the common
    # (cross-core max) chunk counts stay tight.  Dummies sort last, which
    # pins one dummy at the final slot of every core (used as the pad row).
    slot_of_old = np.empty(n, dtype=np.int64)  # slot-major global id
    old_of_slot = np.full(ntot, -1, dtype=np.int64)
    hc = cores // 2
    for side in (0, 1):
        mine = np.where(is_lo_node == (side == 0))[0]
        dl, dhh = d_lo[mine], d_hi[mine]
        run_parity = (dl.max() - dl) % 2  # alternate d_hi dir per d_lo run
        key_hi = np.where(run_parity == 0, -dhh, dhh)
        srt = mine[np.lexsort((key_hi, -dl))]
        assert len(srt) <= hc * npc - hc, (len(srt), npc)
        g = np.arange(len(srt))
        core = side * hc + g % hc
        slot = g // hc
        newids = core * npc + slot
        slot_of_old[srt] = newids
        old_of_slot[newids] = srt

    # slot-major local id s -> row-major local id r = (s%128)*nblk + s//128
    s_loc = np.arange(npc)
    perm_row = (s_loc % blk) * nblk + s_loc // blk  # row id of slot s
    row_of_old = (slot_of_old // npc) * npc + perm_row[slot_of_old % npc]

    # --- per (core, block) lo/hi chunk counts -> common across cores ---
    d_lo_s = np.zeros(ntot, dtype=np.int64)
    d_hi_s = np.zeros(ntot, dtype=np.int64)
    real = old_of_slot >= 0
    d_lo_s[real] = d_lo[old_of_slot[real]]
    d_hi_s[real] = d_hi[old_of_slot[real]]
    kl_cb = d_lo_s.reshape(cores, nblk, blk).max(axis=2)
    kh_cb = d_hi_s.reshape(cores, nblk, blk).max(axis=2)
    K_lo = kl_cb.max(axis=0)  # [nblk] common
    K_hi = kh_cb.max(axis=0)

    off_lo = np.concatenate([[0], np.cumsum(K_lo * blk)])  # slot offsets
    off_hi = np.concatenate([[0], np.cumsum(K_hi * blk)])
    S_lo = int(off_lo[-1])
    S_hi = int(off_hi[-1])

    # pad row: slot npc-1 (the pinned dummy) of core cores/2-1 maps to row
    # npc-1 (p=127, b=nblk-1), i.e. global row half-1 -- same value as the
    # slot-major scheme.
    pad_lo = half - 1
    pad_hi = half - 1  # (value in hi-table local coords: ntot-1-half)

    # --- slot filling ---
    dst_s = slot_of_old[edge_dst]
    src_row = row_of_old[edge_src]
    is_lo = src_row < half

    # rank of each edge within its (dst, class) group
    grp = dst_s * 2 + (~is_lo)
    srt = np.argsort(grp, kind="stable")
    grp_s = grp[srt]
    starts = np.concatenate([[0], np.where(np.diff(grp_s) != 0)[0] + 1])
    group_start = np.zeros(len(grp_s), dtype=np.int64)
    group_start[starts] = starts
    group_start = np.maximum.accumulate(group_start)
    rank_s = np.arange(e) - group_start
    rank = np.empty(e, dtype=np.int64)
    rank[srt] = rank_s

    core_e = dst_s // npc
    blk_e = (dst_s % npc) // blk
    p_e = dst_s % blk

    slots_lo = np.full((cores, S_lo), pad_lo, dtype=np.int16)
    slots_hi = np.full((cores, S_hi), pad_hi, dtype=np.int16)

    lo_m = is_lo
    pos_lo = off_lo[blk_e[lo_m]] + rank[lo_m] * blk + p_e[lo_m]
    slots_lo[core_e[lo_m], pos_lo] = src_row[lo_m].astype(np.int16)
    hi_m = ~is_lo
    pos_hi = off_hi[blk_e[hi_m]] + rank[hi_m] * blk + p_e[hi_m]
    slots_hi[core_e[hi_m], pos_hi] = (src_row[hi_m] - half).astype(np.int16)

    # wrap for dma_gather: element i -> [i%16, i//16], tiled to 128 partitions
    def wrap(a):
        s = a.shape[1]
        if s == 0:
            return np.zeros((cores, 128, 0), dtype=np.int16)
        w = a.reshape(cores, s // 16, 16).transpose(0, 2, 1)  # [cores,16,S/16]
        return np.ascontiguousarray(np.tile(w, (1, 8, 1)))

    # --- group consecutive blocks for merged gathers ---
    groups = []  # list of (b0, b1)  [b0, b1) blocks
    b0 = 0
    ktot = K_lo + K_hi
    while b0 < nblk:
        b1 = b0 + 1
        s = int(ktot[b0])
        while b1 < nblk and s + int(ktot[b1]) <= GK_MAX:
            s += int(ktot[b1])
            b1 += 1
        groups.append((b0, b1))
        b0 = b1

    return dict(
        n=n, e=e, cores=cores, npc=npc, ntot=ntot, half=half, nblk=nblk,
        slot_of_old=slot_of_old, old_of_slot=old_of_slot,
        row_of_old=row_of_old,
        K_lo=K_lo.astype(np.int64), K_hi=K_hi.astype(np.int64),
        off_lo=off_lo, off_hi=off_hi, S_lo=S_lo, S_hi=S_hi,
        idx_lo=wrap(slots_lo), idx_hi=wrap(slots_hi),
        groups=groups,
        util=float(e) / max(1.0, float((S_lo + S_hi) * cores)),
    )


# ---------------------------------------------------------------- program
def build_program(plan, f_in=F_IN, d_hid=D_HID, h=H, n_cls=N_CLS, bf16=True,
                  stop_after=None, repeat=1, mock_cc=False, acc="pe"):
    import concourse.bacc as bacc
    import concourse.mybir as mybir
    from concourse import tile

    dt = mybir.dt
    f32 = dt.float32
    TDT = dt.bfloat16
    AF = mybir.ActivationFunctionType
    ALU = mybir.AluOpType
    dh = d_hid // h
    npc, nblk, half = plan["npc"], plan["nblk"], plan["half"]
    K_lo, K_hi = plan["K_lo"], plan["K_hi"]
    off_lo, off_hi = plan["off_lo"], plan["off_hi"]
    S_lo, S_hi = plan["S_lo"], plan["S_hi"]
    cores = plan["cores"]
    ntot = plan["ntot"]
    ROW = 128  # table row, elements (256B in bf16 -- dma_gather granule)
    DCAT = d_hid + 2 * h  # 112: [h | alpha_src | alpha_dst]
    ZCOL = d_hid + h  # 104: columns accumulated in PSUM (msg | s_exp)

    nc = bacc.Bacc("TRN2", target_bir_lowering=False, debug=False,
                   num_devices=cores)

    # ---- I/O ----
    xT = nc.dram_tensor("xT", [f_in, npc], TDT, kind="ExternalInput")
    W0cat = nc.dram_tensor("W0cat", [f_in, DCAT], TDT, kind="ExternalInput")
    W1cat = nc.dram_tensor("W1cat", [d_hid, DCAT], TDT, kind="ExternalInput")
    Wout = nc.dram_tensor("Wout", [d_hid, n_cls], TDT, kind="ExternalInput")
    b0b = nc.dram_tensor("b0b", [128, d_hid], f32, kind="ExternalInput")
    b1b = nc.dram_tensor("b1b", [128, d_hid], f32, kind="ExternalInput")
    boutb = nc.dram_tensor("boutb", [128, n_cls], f32, kind="ExternalInput")
    identt = nc.dram_tensor("identt", [128, 128], TDT, kind="ExternalInput")
    idx_lo_d = nc.dram_tensor("idx_lo", [128, max(S_lo // 16, 1)], dt.int16,
                              kind="ExternalInput")
    idx_hi_d = nc.dram_tensor("idx_hi", [128, max(S_hi // 16, 1)], dt.int16,
                              kind="ExternalInput")
    out_d = nc.dram_tensor("out", [npc, n_cls], f32, kind="ExternalOutput")

    with tile.TileContext(nc) as tc:
        with (
            tc.tile_pool(name="dram", bufs=1, space="DRAM") as dramp,
            tc.tile_pool(name="persist", bufs=1) as pers,
            tc.tile_pool(name="gath", bufs=4) as gath,
            tc.tile_pool(name="sexp", bufs=3) as sexp,
            tc.tile_pool(name="stage", bufs=4) as stage,
            tc.tile_pool(name="small", bufs=4) as small,
            tc.tile_pool(name="psA", bufs=2, space="PSUM") as psA,
            tc.tile_pool(name="psB", bufs=2, space="PSUM") as psB,
            tc.tile_pool(name="psT", bufs=2, space="PSUM") as psT,
        ):
            # ---- DRAM scratch ----
            tslice = dramp.tile([npc, ROW], TDT)
            aspace = "Local" if mock_cc else "Shared"
            tfulls = [
                (dramp.tile([ntot, ROW], TDT, addr_space=aspace,
                            name=f"tf0_{r}", tag=f"tf0_{r}"),
                 dramp.tile([ntot, ROW], TDT, addr_space=aspace,
                            name=f"tf1_{r}", tag=f"tf1_{r}"))
                for r in range(repeat)
            ]

            # ---- persistent SBUF ----
            xT_sb = pers.tile([f_in, npc], TDT)
            nc.sync.dma_start(xT_sb[:], xT[:, :])
            W0_sb = pers.tile([f_in, DCAT], TDT)
            nc.sync.dma_start(W0_sb[:], W0cat[:, :])
            W1_sb = pers.tile([d_hid, DCAT], TDT)
            nc.sync.dma_start(W1_sb[:], W1cat[:, :])
            Wo_sb = pers.tile([d_hid, n_cls], TDT)
            nc.sync.dma_start(Wo_sb[:], Wout[:, :])
            b0_sb = pers.tile([128, d_hid], f32)
            nc.sync.dma_start(b0_sb[:], b0b[:, :])
            b1_sb = pers.tile([128, d_hid], f32)
            nc.sync.dma_start(b1_sb[:], b1b[:, :])
            bo_sb = pers.tile([128, n_cls], f32)
            nc.sync.dma_start(bo_sb[:], boutb[:, :])
            idt_sb = pers.tile([128, 128], TDT)
            nc.sync.dma_start(idt_sb[:], identt[:, :])
            if S_lo:
                ixlo_sb = pers.tile([128, S_lo // 16], dt.int16)
                nc.sync.dma_start(ixlo_sb[:], idx_lo_d[:, :])
            if S_hi:
                ixhi_sb = pers.tile([128, S_hi // 16], dt.int16)
                nc.sync.dma_start(ixhi_sb[:], idx_hi_d[:, :])
            table0_sb = pers.tile([128, nblk * ROW], TDT)
            table1_sb = pers.tile([128, nblk * ROW], TDT)
            h1_sb = pers.tile([128, nblk * d_hid], TDT)
            h2_sb = pers.tile([128, nblk * d_hid], TDT)
            out_sb = pers.tile([128, nblk * n_cls], f32)
            padrow = pers.tile([1, h], TDT)
            nc.vector.memset(padrow[:], PAD_AS)
            # pad cols (DCAT:ROW) are shipped by the contiguous table DMA;
            # zero them once so no uninitialized bytes flow
            for tbl in (table0_sb, table1_sb):
                nc.vector.memset(
                    tbl[:, :].rearrange("p (b e) -> p b e",
                                        e=ROW)[:, :, DCAT:ROW], 0.0)

            # ================= helper: table build =================
            def table_build(src_lhsT, Wc_sb, table_sb, tf):
                """src_lhsT(b) -> lhsT AP [k, 128] for block b."""
                for b in range(nblk):
                    lhsT = src_lhsT(b)
                    ps = psA.tile([128, DCAT], mybir.dt.float32)
                    nc.tensor.matmul(ps[:], lhsT, Wc_sb[:, :], start=True,
                                     stop=True)
                    # f32 PSUM -> bf16 table tile on the idle ACT engine
                    nc.scalar.activation(
                        table_sb[:, b * ROW:b * ROW + DCAT],
                        ps[:, 0:DCAT], AF.Copy)
                # one contiguous DMA: row r = p*nblk + b  <=>  [p, (b e)]
                nc.sync.dma_start(
                    tslice[:, :].rearrange("(p b) e -> p (b e)", p=128),
                    table_sb[:, :])
                # pad row: overwrite alpha_src of row npc-1 (pinned dummy)
                nc.sync.dma_start(
                    tslice[npc - 1:npc, d_hid:d_hid + h], padrow[:])
                if mock_cc:
                    # cost-model stand-in: move the same bytes the AllGather
                    # would receive (cores-1 slices in + 1 local copy)
                    for c in range(cores):
                        nc.sync.dma_start(
                            tf[c * npc:(c + 1) * npc, :], tslice[:, :])
                else:
                    nc.gpsimd.collective_compute(
                        "AllGather", mybir.AluOpType.bypass,
                        replica_groups=[list(range(cores))],
                        ins=[tslice[:, :]], outs=[tf[:, :]])

            # ================= helper: edge phase =================
            def edge_phase(tf, table_sb, post):
                """post(b, ps) consumes psum [128, ZCOL] for block b.

                Gathers are merged across groups of consecutive blocks to
                amortize the ~1us SWDGE fixed cost per dma_gather; edge math
                runs group-wide where it is block-agnostic.
                """
                for (g0, g1) in plan["groups"]:
                    KLg = int(off_lo[g1] - off_lo[g0]) // 128
                    KHg = int(off_hi[g1] - off_hi[g0]) // 128
                    Kg = KLg + KHg
                    if Kg == 0:
                        for b in range(g0, g1):
                            post(b, None)
                        continue
                    G = gath.tile([128, GK_MAX * 128], TDT, tag="G")
                    if KLg:
                        nc.gpsimd.dma_gather(
                            G[:, :KLg * 128].rearrange("p (k e) -> p k e",
                                                       e=128),
                            tf[0:half, :],
                            ixlo_sb[:, off_lo[g0] // 16:off_lo[g1] // 16],
                            128 * KLg, 128 * KLg, ROW,
                            single_packet=False)
                    if KHg:
                        nc.gpsimd.dma_gather(
                            G[:, KLg * 128:Kg * 128].rearrange(
                                "p (k e) -> p k e", e=128),
                            tf[half:ntot, :],
                            ixhi_sb[:, off_hi[g0] // 16:off_hi[g1] // 16],
                            128 * KHg, 128 * KHg, ROW,
                            single_packet=False)
                    Gv = G[:, :Kg * 128].rearrange("p (k e) -> p k e", e=128)

                    def blk_ranges(b):
                        lo = ((off_lo[b] - off_lo[g0]) // 128,
                              (off_lo[b + 1] - off_lo[g0]) // 128)
                        hi = (KLg + (off_hi[b] - off_hi[g0]) // 128,
                              KLg + (off_hi[b + 1] - off_hi[g0]) // 128)
                        return [r for r in (lo, hi) if r[1] > r[0]]

                    # e = alpha_src + alpha_dst (per block: ad varies); the
                    # local table tile holds alpha_dst at cols ZCOL:DCAT
                    for b in range(g0, g1):
                        adc = table_sb[:, b * ROW + ZCOL:b * ROW + DCAT]
                        for (c0, c1) in blk_ranges(b):
                            kk = int(c1 - c0)
                            nc.vector.tensor_add(
                                Gv[:, c0:c1, DCAT:DCAT + h],
                                Gv[:, c0:c1, d_hid:d_hid + h],
                                adc.rearrange("p (o j) -> p o j",
                                              o=1).broadcast_to([128, kk, h]))
                    # exp(leakyrelu(e)) == exp(max(e, 0.2e)) -- group-wide
                    ev = Gv[:, :, DCAT:DCAT + h]
                    nc.vector.scalar_tensor_tensor(
                        ev, ev, NEG_SLOPE, ev, op0=ALU.mult, op1=ALU.max)
                    # 12-wide expanded weights (packed) for the message mul
                    SE = sexp.tile([128, GK_MAX * d_hid], TDT, tag="SE")
                    SEv = SE[:, :Kg * d_hid].rearrange(
                        "p (k j d) -> p k j d", j=h, d=dh)
                    nc.scalar.activation(
                        SEv,
                        ev.rearrange("p k (j o) -> p k j o",
                                     o=1).broadcast_to([128, Kg, h, dh]),
                        AF.Exp)
                    # 8-wide weights into cols d_hid:ZCOL (accumulated as the
                    # softmax denominator); strided copy of lane 0 of each
                    # head from SE.  Overwrites gathered alpha_src, which the
                    # e-adds above already consumed.
                    nc.vector.tensor_copy(
                        Gv[:, :, d_hid:d_hid + h].rearrange(
                            "p k (j o) -> p k j o", o=1),
                        SE[:, :Kg * d_hid].rearrange(
                            "p (k j d) -> p k j d", j=h, d=dh)[:, :, :, 0:1])
                    # fully-packed bf16 multiply (2x DVE mode)
                    nc.vector.tensor_mul(
                        Gv[:, :, 0:d_hid], Gv[:, :, 0:d_hid],
                        SE[:, :Kg * d_hid].rearrange("p (k f) -> p k f",
                                                     f=d_hid))
                    # per-block accumulate [msg | s_exp] via identity matmul
                    for b in range(g0, g1):
                        chunks = [c for (c0, c1) in blk_ranges(b)
                                  for c in range(c0, c1)]
                        if not chunks:
                            post(b, None)
                            continue
                        ps = psB.tile([128, ZCOL], mybir.dt.float32)
                        for i, c in enumerate(chunks):
                            nc.tensor.matmul(
                                ps[:], idt_sb[:, :],
                                G[:, c * 128:c * 128 + ZCOL],
                                start=(i == 0), stop=(i == len(chunks) - 1))
                        post(b, ps)

            # ======= helper: output MLP + log_softmax (batched blocks) =====
            # Per block: transpose + matmul into a shared multi-block PSUM
            # tile.  Every OUT_B blocks one batched epilogue computes
            # out = t0 - ln(sum(exp(t0))) over [128, OUT_B*n_cls] at once
            # (the max-shift is skipped: logits are O(1)).
            OUT_B = 7
            ostate = {"po": None, "b0": 0, "cnt": 0}

            def out_flush():
                nb, po = ostate["cnt"], ostate["po"]
                if not nb:
                    return
                b0 = ostate["b0"]
                w = nb * n_cls
                t0 = stage.tile([128, OUT_B * n_cls], mybir.dt.float32,
                                tag="t0")
                nc.vector.tensor_add(
                    t0[:, 0:w].rearrange("p (b c) -> p b c", c=n_cls),
                    po[:, 0:w].rearrange("p (b c) -> p b c", c=n_cls),
                    bo_sb[:, :].rearrange("p (o c) -> p o c",
                                          o=1).broadcast_to([128, nb, n_cls]))
                ex = stage.tile([128, OUT_B * n_cls], mybir.dt.float32,
                                tag="ex")
                nc.scalar.activation(ex[:, 0:w], t0[:, 0:w], AF.Exp)
                s = small.tile([128, OUT_B], mybir.dt.float32, tag="s")
                nc.vector.reduce_sum(
                    s[:, 0:nb], ex[:, 0:w].rearrange("p (b c) -> p b c",
                                                     c=n_cls),
                    axis=mybir.AxisListType.X)
                ls = small.tile([128, OUT_B], mybir.dt.float32, tag="ls")
                nc.scalar.activation(ls[:, 0:nb], s[:, 0:nb], AF.Ln)
                nc.vector.tensor_sub(
                    out_sb[:, b0 * n_cls:b0 * n_cls + w].rearrange(
                        "p (b c) -> p b c", c=n_cls),
                    t0[:, 0:w].rearrange("p (b c) -> p b c", c=n_cls),
                    ls[:, 0:nb].rearrange("p (b o) -> p b o",
                                          o=1).broadcast_to(
                        [128, nb, n_cls]))
                ostate["po"] = None
                ostate["cnt"] = 0

            def out_block(b, hv):
                """hv: [128, d_hid] bf16 SBUF view of layer-2 activations."""
                pst = psT.tile([d_hid, 128], TDT, tag="ptr")
                nc.tensor.transpose(pst[:], hv, idt_sb[:, :])
                hT = stage.tile([d_hid, 128], TDT, tag="hT")
                nc.vector.tensor_copy(hT[:], pst[:])
                if ostate["po"] is None:
                    po7 = psT.tile([128, OUT_B * n_cls], mybir.dt.float32,
                                   tag="po", name="po7")
                    ostate["po"] = po7
                    ostate["b0"] = b
                i = ostate["cnt"]
                nc.tensor.matmul(
                    ostate["po"][:, i * n_cls:(i + 1) * n_cls],
                    hT[:, :], Wo_sb[:, :], start=True, stop=True)
                ostate["cnt"] = i + 1
                if ostate["cnt"] == OUT_B:
                    out_flush()

            bailed = False

            # ================= phase A: table 0 =================
            for _rep in range(repeat):
              tfull0, tfull1 = tfulls[_rep]
              table_build(
                  lambda b: xT_sb[:, b * 128:(b + 1) * 128],
                  W0_sb, table0_sb, tfull0)

              if stop_after == "A":
                  bailed = True

              # ================= phase B: layer-0 edges =================
              def post0(b, ps):
                  hv = h1_sb[:, b * d_hid:(b + 1) * d_hid]
                  if ps is None:
                      nc.vector.tensor_copy(hv, b0_sb[:, :])
                      return
                  z = small.tile([128, h], mybir.dt.float32, tag="z")
                  nc.vector.tensor_scalar_add(z[:], ps[:, d_hid:ZCOL], 1e-16)
                  iz = small.tile([128, h], mybir.dt.float32, tag="iz")
                  nc.vector.reciprocal(iz[:], z[:])
                  izb = iz[:, :].rearrange("p (j o) -> p j o",
                                           o=1).broadcast_to([128, h, dh])
                  hv3 = hv.rearrange("p (j d) -> p j d", d=dh)
                  nc.vector.tensor_mul(hv3, ps[:, 0:d_hid].rearrange(
                      "p (j d) -> p j d", d=dh), izb)
                  nc.vector.tensor_add(hv, hv, b0_sb[:, :])

              if not bailed:
                  edge_phase(tfull0, table0_sb, post0)
              if stop_after == "B":
                  bailed = True

              # ================= phase C: table 1 =================
              def lhsT1(b):
                  pst = psT.tile([d_hid, 128], TDT, tag="ptr")
                  nc.tensor.transpose(
                      pst[:], h1_sb[:, b * d_hid:(b + 1) * d_hid],
                      idt_sb[:, :])
                  hT = stage.tile([d_hid, 128], TDT, tag="hT")
                  nc.vector.tensor_copy(hT[:], pst[:])
                  return hT[:, :]

              if not bailed:
                  table_build(lhsT1, W1_sb, table1_sb, tfull1)
              if stop_after == "C":
                  bailed = True

              # ======== phase D: layer-1 edges + fused output MLP ========
              def post1(b, ps):
                  hv = h2_sb[:, b * d_hid:(b + 1) * d_hid]
                  if ps is None:
                      t = small.tile([128, d_hid], mybir.dt.float32, tag="t1")
                      nc.vector.tensor_copy(t[:], b1_sb[:, :])
                      nc.vector.tensor_scalar_max(hv, t[:], 0.0)
                      out_block(b, hv)
                      return
                  z = small.tile([128, h], mybir.dt.float32, tag="z")
                  nc.vector.tensor_scalar_add(z[:], ps[:, d_hid:ZCOL], 1e-16)
                  iz = small.tile([128, h], mybir.dt.float32, tag="iz")
                  nc.vector.reciprocal(iz[:], z[:])
                  izb = iz[:, :].rearrange("p (j o) -> p j o",
                                           o=1).broadcast_to([128, h, dh])
                  t = small.tile([128, d_hid], mybir.dt.float32, tag="t1")
                  t3 = t[:, :].rearrange("p (j d) -> p j d", d=dh)
                  nc.vector.tensor_mul(t3, ps[:, 0:d_hid].rearrange(
                      "p (j d) -> p j d", d=dh), izb)
                  nc.vector.tensor_add(t[:], t[:], b1_sb[:, :])
                  nc.vector.tensor_scalar_max(hv, t[:], 0.0)  # ReLU
                  out_block(b, hv)

              if not bailed:
                  edge_phase(tfull1, table1_sb, post1)
                  out_flush()

            if bailed:
                nc.vector.memset(out_sb[:, :], 0.0)
            # one contiguous DMA: out row r = p*nblk + b  <=>  [p, (b c)]
            nc.sync.dma_start(
                out_d[:, :].rearrange("(p b) c -> p (b c)", p=128),
                out_sb[:, :])

    nc.compile()
    return nc


# ---------------------------------------------------------------- inputs
def make_in_maps(plan, inputs, f_in=F_IN, d_hid=D_HID, h=H, n_cls=N_CLS,
                 bf16=True):
    import ml_dtypes

    x = np.asarray(inputs["x"], dtype=np.float32)
    W0 = np.asarray(inputs["W0"], dtype=np.float32)
    W1 = np.asarray(inputs["W1"], dtype=np.float32)
    Wout = np.asarray(inputs["Wout"], dtype=np.float32)
    as0 = np.asarray(inputs["as0"], dtype=np.float32)
    ad0 = np.asarray(inputs["ad0"], dtype=np.float32)
    as1 = np.asarray(inputs["as1"], dtype=np.float32)
    ad1 = np.asarray(inputs["ad1"], dtype=np.float32)
    b0 = np.asarray(inputs["b0"], dtype=np.float32)
    b1 = np.asarray(inputs["b1"], dtype=np.float32)
    bout = np.asarray(inputs["bout"], dtype=np.float32)

    dh = d_hid // h
    npc, cores = plan["npc"], plan["cores"]
    old_of_slot = plan["old_of_slot"]

    def blockdiag(a):  # [h, dh] -> [d_hid, h]
        m = np.zeros((d_hid, h), dtype=np.float32)
        for j in range(h):
            m[j * dh:(j + 1) * dh, j] = a[j]
        return m

    W0cat = np.concatenate(
        [W0, W0 @ blockdiag(as0), W0 @ blockdiag(ad0)], axis=1)
    W1cat = np.concatenate(
        [W1, W1 @ blockdiag(as1), W1 @ blockdiag(ad1)], axis=1)

    tdt = ml_dtypes.bfloat16
    ident = np.eye(128, dtype=tdt)
    b0b = np.ascontiguousarray(np.broadcast_to(b0, (128, d_hid)))
    b1b = np.ascontiguousarray(np.broadcast_to(b1, (128, d_hid)))
    boutb = np.ascontiguousarray(np.broadcast_to(bout, (128, n_cls)))

    x_ext = np.zeros((npc * cores, f_in), dtype=np.float32)
    real = old_of_slot >= 0
    x_ext[real] = x[old_of_slot[real]]

    in_maps = []
    for c in range(cores):
        xs = x_ext[c * npc:(c + 1) * npc]
        m = dict(
            xT=np.ascontiguousarray(xs.T).astype(tdt),
            W0cat=W0cat.astype(tdt), W1cat=W1cat.astype(tdt),
            Wout=Wout.astype(tdt),
            b0b=b0b, b1b=b1b, boutb=boutb,
            identt=ident,
            idx_lo=np.ascontiguousarray(plan["idx_lo"][c])
            if plan["S_lo"] else np.zeros((128, 1), np.int16),
            idx_hi=np.ascontiguousarray(plan["idx_hi"][c])
            if plan["S_hi"] else np.zeros((128, 1), np.int16),
        )
        in_maps.append(m)
    return in_maps


def assemble_output(plan, results, n_cls=N_CLS):
    outs = np.concatenate([r["out"] for r in results], axis=0)
    return np.ascontiguousarray(outs[plan["row_of_old"]], dtype=np.float32)


# ---------------------------------------------------------------- entry
def kernel(**inputs):
    from concourse.bass_utils import run_bass_kernel_spmd

    edge_src = np.asarray(inputs["edge_src"]).astype(np.int64)
    edge_dst = np.asarray(inputs["edge_dst"]).astype(np.int64)

    bf16 = True
    key = (edge_src.tobytes(), edge_dst.tobytes(), bf16)
    kh = hash(key)
    if kh not in _CACHE:
        plan = make_plan(edge_src, edge_dst)
        nc = build_program(plan, bf16=bf16)
        _CACHE[kh] = (plan, nc)
    plan, nc = _CACHE[kh]

    in_maps = make_in_maps(plan, inputs, bf16=bf16)
    res = run_bass_kernel_spmd(nc, in_maps,
                               core_ids=list(range(plan["cores"])))
    return assemble_output(plan, res.results)


# revision 26
# speedup vs baseline: 1.5123x; 1.0308x over previous
"""GAT (2-layer graph attention network + output MLP) on 8 Trainium2 NeuronCores.

Strategy ("diagonal scheduling", v2):
  - The Bass program is built per-invocation, so the graph structure is a
    compile-time constant.  Nodes are assigned to cores balancing total
    in-degree, and within each core nodes are grouped into 128-node blocks
    sorted by (deg_lo, deg_hi) so that all nodes in a block have nearly equal
    in-degree from each half of the node space.
  - Edges of a block are laid out in "chunks" of 128 slots: slot (c, p) holds
    the c-th in-edge of the node on partition p.  A chunk therefore has at
    most one edge per destination, which turns the segment-softmax scatter
    into a plain PSUM accumulation with a constant identity stationary matrix
    (no masks, no segment ops).
  - Per layer, each core computes the feature/attention table rows for its own
    nodes ([h | alpha_src | alpha_dst] per node), all-gathers the full table to
    DRAM, and then gathers per-edge rows with dma_gather (int16 indices force
    a lo/hi table split at NTOT/2).
  - Table rows are numbered partition-major (row = p*nblk + b) so the whole
    per-core table slice is written to DRAM with ONE contiguous DMA straight
    from the SBUF-resident table tile (which also serves as the local
    alpha_dst source).  The PSUM->SBUF bf16 convert runs on the otherwise-idle
    Activation engine.
  - softmax: exp(leakyrelu(e)) == exp(max(e, 0.2e)) -- one DVE
    scalar_tensor_tensor (mult/max), then two ACT exps: an 8-wide one (the
    per-edge weights accumulated for the softmax denominator) and a 12-wide
    broadcast-EXPANDED one so the message multiply is a fully-packed bf16
    TensorTensor (2x DVE mode).  The 1/z normalization folds in after
    aggregation (exact same math as the reference; max-subtraction is skipped
    since logits are O(1)).
  - The output MLP + log_softmax is folded into layer-1's per-block epilogue
    and accumulated in SBUF; one contiguous DMA writes the result at the end.

kernel(**inputs) -> np.ndarray  takes full inputs, returns the full output.
"""

import numpy as np

# ---------------------------------------------------------------- constants
N, E, F_IN, D_HID, H, N_CLS = 50000, 800000, 128, 96, 8, 40
DH = D_HID // H  # 12
NEG_SLOPE = 0.2
CORES = 8
BLK = 128
PAD_AS = -10000.0  # alpha_src for pad rows: exp(leaky(PAD_AS+ad)) == 0.0
GK_MAX = 64  # max chunks per merged-gather group

_CACHE = {}


# ---------------------------------------------------------------- planning
def make_plan(edge_src, edge_dst, n=N, cores=CORES, blk=BLK):
    """Pure graph-structure planning (numpy only).

    Returns a dict with the node permutations, per-block common chunk counts
    and the per-core wrapped int16 gather-index arrays.  Node ids come in two
    numberings: "slot-major" (slot s = b*128 + p; used for the xT input
    layout) and "row-major" (table row r = p*nblk + b; used for the DRAM
    table, the gather indices and the output layout).
    """
    edge_src = np.asarray(edge_src, dtype=np.int64)
    edge_dst = np.asarray(edge_dst, dtype=np.int64)
    e = len(edge_src)

    deg = np.bincount(edge_dst, minlength=n)

    # nodes per core, including dummies; one dummy pinned last on every core
    npc = -(-(n + cores) // (cores * blk)) * blk  # round up to block multiple
    ntot = cores * npc
    half = ntot // 2
    nblk = npc // blk

    # --- assign real nodes to cores balancing total degree (snake deal) ---
    order = np.argsort(-deg, kind="stable")  # real nodes by degree desc
    core_of = np.empty(n, dtype=np.int64)
    r = np.arange(n)
    rnd = r // cores
    pos = r % cores
    fwd = (rnd % 2) == 0
    lane = np.where(fwd, pos, cores - 1 - pos)
    core_of[order] = lane

    # lo set = cores 0..cores/2-1
    is_lo_node = core_of < (cores // 2)
    src_is_lo = is_lo_node[edge_src]
    d_lo = np.bincount(edge_dst[src_is_lo], minlength=n)
    d_hi = deg - d_lo

    # --- per-half global ordering, dealt round-robin to the half's cores ---
    # Sorting each half globally by (d_lo desc, d_hi snake) and dealing node
    # at sorted position g to core g%hc, slot g//hc keeps every core's block
    # profile an interleaved sample of the same distribution, so the common
    # (cross-core max) chunk counts stay tight.  Dummies sort last, which
    # pins one dummy at the final slot of every core (used as the pad row).
    slot_of_old = np.empty(n, dtype=np.int64)  # slot-major global id
    old_of_slot = np.full(ntot, -1, dtype=np.int64)
    hc = cores // 2
    for side in (0, 1):
        mine = np.where(is_lo_node == (side == 0))[0]
        dl, dhh = d_lo[mine], d_hi[mine]
        run_parity = (dl.max() - dl) % 2  # alternate d_hi dir per d_lo run
        key_hi = np.where(run_parity == 0, -dhh, dhh)
        srt = mine[np.lexsort((key_hi, -dl))]
        assert len(srt) <= hc * npc - hc, (len(srt), npc)
        g = np.arange(len(srt))
        core = side * hc + g % hc
        slot = g // hc
        newids = core * npc + slot
        slot_of_old[srt] = newids
        old_of_slot[newids] = srt

    # slot-major local id s -> row-major local id r = (s%128)*nblk + s//128
    s_loc = np.arange(npc)
    perm_row = (s_loc % blk) * nblk + s_loc // blk  # row id of slot s
    row_of_old = (slot_of_old // npc) * npc + perm_row[slot_of_old % npc]

    # --- per (core, block) lo/hi chunk counts -> common across cores ---
    d_lo_s = np.zeros(ntot, dtype=np.int64)
    d_hi_s = np.zeros(ntot, dtype=np.int64)
    real = old_of_slot >= 0
    d_lo_s[real] = d_lo[old_of_slot[real]]
    d_hi_s[real] = d_hi[old_of_slot[real]]
    kl_cb = d_lo_s.reshape(cores, nblk, blk).max(axis=2)
    kh_cb = d_hi_s.reshape(cores, nblk, blk).max(axis=2)
    K_lo = kl_cb.max(axis=0)  # [nblk] common
    K_hi = kh_cb.max(axis=0)

    off_lo = np.concatenate([[0], np.cumsum(K_lo * blk)])  # slot offsets
    off_hi = np.concatenate([[0], np.cumsum(K_hi * blk)])
    S_lo = int(off_lo[-1])
    S_hi = int(off_hi[-1])

    # pad row: slot npc-1 (the pinned dummy) of core cores/2-1 maps to row
    # npc-1 (p=127, b=nblk-1), i.e. global row half-1 -- same value as the
    # slot-major scheme.
    pad_lo = half - 1
    pad_hi = half - 1  # (value in hi-table local coords: ntot-1-half)

    # --- slot filling ---
    dst_s = slot_of_old[edge_dst]
    src_row = row_of_old[edge_src]
    is_lo = src_row < half

    # rank of each edge within its (dst, class) group
    grp = dst_s * 2 + (~is_lo)
    srt = np.argsort(grp, kind="stable")
    grp_s = grp[srt]
    starts = np.concatenate([[0], np.where(np.diff(grp_s) != 0)[0] + 1])
    group_start = np.zeros(len(grp_s), dtype=np.int64)
    group_start[starts] = starts
    group_start = np.maximum.accumulate(group_start)
    rank_s = np.arange(e) - group_start
    rank = np.empty(e, dtype=np.int64)
    rank[srt] = rank_s

    core_e = dst_s // npc
    blk_e = (dst_s % npc) // blk
    p_e = dst_s % blk

    slots_lo = np.full((cores, S_lo), pad_lo, dtype=np.int16)
    slots_hi = np.full((cores, S_hi), pad_hi, dtype=np.int16)

    lo_m = is_lo
    pos_lo = off_lo[blk_e[lo_m]] + rank[lo_m] * blk + p_e[lo_m]
    slots_lo[core_e[lo_m], pos_lo] = src_row[lo_m].astype(np.int16)
    hi_m = ~is_lo
    pos_hi = off_hi[blk_e[hi_m]] + rank[hi_m] * blk + p_e[hi_m]
    slots_hi[core_e[hi_m], pos_hi] = (src_row[hi_m] - half).astype(np.int16)

    # wrap for dma_gather: element i -> [i%16, i//16], tiled to 128 partitions
    def wrap(a):
        s = a.shape[1]
        if s == 0:
            return np.zeros((cores, 128, 0), dtype=np.int16)
        w = a.reshape(cores, s // 16, 16).transpose(0, 2, 1)  # [cores,16,S/16]
        return np.ascontiguousarray(np.tile(w, (1, 8, 1)))

    # --- group consecutive blocks for merged gathers ---
    # the last TAIL_SINGLES blocks get singleton groups so the post-gather
    # compute drain at the end of each edge phase is short
    TAIL_SINGLES = 2
    groups = []  # list of (b0, b1)  [b0, b1) blocks
    b0 = 0
    ktot = K_lo + K_hi
    while b0 < nblk:
        if nblk - b0 <= TAIL_SINGLES:
            groups.append((b0, b0 + 1))
            b0 += 1
            continue
        b1 = b0 + 1
        s = int(ktot[b0])
        while (b1 < nblk and s + int(ktot[b1]) <= GK_MAX
               and nblk - b1 > TAIL_SINGLES):
            s += int(ktot[b1])
            b1 += 1
        groups.append((b0, b1))
        b0 = b1

    return dict(
        n=n, e=e, cores=cores, npc=npc, ntot=ntot, half=half, nblk=nblk,
        slot_of_old=slot_of_old, old_of_slot=old_of_slot,
        row_of_old=row_of_old,
        K_lo=K_lo.astype(np.int64), K_hi=K_hi.astype(np.int64),
        off_lo=off_lo, off_hi=off_hi, S_lo=S_lo, S_hi=S_hi,
        idx_lo=wrap(slots_lo), idx_hi=wrap(slots_hi),
        groups=groups,
        util=float(e) / max(1.0, float((S_lo + S_hi) * cores)),
    )


# ---------------------------------------------------------------- program
def build_program(plan, f_in=F_IN, d_hid=D_HID, h=H, n_cls=N_CLS, bf16=True,
                  stop_after=None, repeat=1, mock_cc=False, acc="pe"):
    import concourse.bacc as bacc
    import concourse.mybir as mybir
    from concourse import tile

    dt = mybir.dt
    f32 = dt.float32
    TDT = dt.bfloat16
    AF = mybir.ActivationFunctionType
    ALU = mybir.AluOpType
    dh = d_hid // h
    npc, nblk, half = plan["npc"], plan["nblk"], plan["half"]
    K_lo, K_hi = plan["K_lo"], plan["K_hi"]
    off_lo, off_hi = plan["off_lo"], plan["off_hi"]
    S_lo, S_hi = plan["S_lo"], plan["S_hi"]
    cores = plan["cores"]
    ntot = plan["ntot"]
    ROW = 128  # table row, elements (256B in bf16 -- dma_gather granule)
    DCAT = d_hid + 2 * h  # 112: [h | alpha_src | alpha_dst]
    ZCOL = d_hid + h  # 104: columns accumulated in PSUM (msg | s_exp)

    nc = bacc.Bacc("TRN2", target_bir_lowering=False, debug=False,
                   num_devices=cores)

    # ---- I/O ----
    xT = nc.dram_tensor("xT", [f_in, npc], TDT, kind="ExternalInput")
    W0cat = nc.dram_tensor("W0cat", [f_in, DCAT], TDT, kind="ExternalInput")
    W1cat = nc.dram_tensor("W1cat", [d_hid, DCAT], TDT, kind="ExternalInput")
    Wout = nc.dram_tensor("Wout", [d_hid, n_cls], TDT, kind="ExternalInput")
    b0b = nc.dram_tensor("b0b", [128, d_hid], f32, kind="ExternalInput")
    b1b = nc.dram_tensor("b1b", [128, d_hid], f32, kind="ExternalInput")
    boutb = nc.dram_tensor("boutb", [128, n_cls], f32, kind="ExternalInput")
    identt = nc.dram_tensor("identt", [128, 128], TDT, kind="ExternalInput")
    idx_lo_d = nc.dram_tensor("idx_lo", [128, max(S_lo // 16, 1)], dt.int16,
                              kind="ExternalInput")
    idx_hi_d = nc.dram_tensor("idx_hi", [128, max(S_hi // 16, 1)], dt.int16,
                              kind="ExternalInput")
    out_d = nc.dram_tensor("out", [npc, n_cls], f32, kind="ExternalOutput")

    with tile.TileContext(nc) as tc:
        with (
            tc.tile_pool(name="dram", bufs=1, space="DRAM") as dramp,
            tc.tile_pool(name="persist", bufs=1) as pers,
            tc.tile_pool(name="gath", bufs=4) as gath,
            tc.tile_pool(name="sexp", bufs=3) as sexp,
            tc.tile_pool(name="stage", bufs=4) as stage,
            tc.tile_pool(name="small", bufs=4) as small,
            tc.tile_pool(name="psA", bufs=2, space="PSUM") as psA,
            tc.tile_pool(name="psB", bufs=3, space="PSUM") as psB,
            tc.tile_pool(name="psT", bufs=2, space="PSUM") as psT,
            tc.tile_pool(name="psO", bufs=1, space="PSUM") as psO,
        ):
            # ---- DRAM scratch ----
            tslice = dramp.tile([npc, ROW], TDT)
            aspace = "Local" if mock_cc else "Shared"
            tfulls = [
                (dramp.tile([ntot, ROW], TDT, addr_space=aspace,
                            name=f"tf0_{r}", tag=f"tf0_{r}"),
                 dramp.tile([ntot, ROW], TDT, addr_space=aspace,
                            name=f"tf1_{r}", tag=f"tf1_{r}"))
                for r in range(repeat)
            ]

            # ---- persistent SBUF ----
            xT_sb = pers.tile([f_in, npc], TDT)
            nc.sync.dma_start(xT_sb[:], xT[:, :])
            W0_sb = pers.tile([f_in, DCAT], TDT)
            nc.sync.dma_start(W0_sb[:], W0cat[:, :])
            W1_sb = pers.tile([d_hid, DCAT], TDT)
            nc.sync.dma_start(W1_sb[:], W1cat[:, :])
            Wo_sb = pers.tile([d_hid, n_cls], TDT)
            nc.sync.dma_start(Wo_sb[:], Wout[:, :])
            b0_sb = pers.tile([128, d_hid], f32)
            nc.sync.dma_start(b0_sb[:], b0b[:, :])
            b1_sb = pers.tile([128, d_hid], f32)
            nc.sync.dma_start(b1_sb[:], b1b[:, :])
            bo_sb = pers.tile([128, n_cls], f32)
            nc.sync.dma_start(bo_sb[:], boutb[:, :])
            idt_sb = pers.tile([128, 128], TDT)
            nc.sync.dma_start(idt_sb[:], identt[:, :])
            if S_lo:
                ixlo_sb = pers.tile([128, S_lo // 16], dt.int16)
                nc.sync.dma_start(ixlo_sb[:], idx_lo_d[:, :])
            if S_hi:
                ixhi_sb = pers.tile([128, S_hi // 16], dt.int16)
                nc.sync.dma_start(ixhi_sb[:], idx_hi_d[:, :])
            table0_sb = pers.tile([128, nblk * ROW], TDT)
            table1_sb = pers.tile([128, nblk * ROW], TDT)
            h1_sb = pers.tile([128, nblk * d_hid], TDT)
            h2_sb = pers.tile([128, nblk * d_hid], TDT)
            out_sb = pers.tile([128, nblk * n_cls], f32)
            s_all = pers.tile([128, nblk], f32)
            padrow = pers.tile([1, h], TDT)
            nc.vector.memset(padrow[:], PAD_AS)
            # pad cols (DCAT:ROW) are shipped by the contiguous table DMA;
            # zero them once so no uninitialized bytes flow
            for tbl in (table0_sb, table1_sb):
                nc.vector.memset(
                    tbl[:, :].rearrange("p (b e) -> p b e",
                                        e=ROW)[:, :, DCAT:ROW], 0.0)

            # ================= helpers: table build/ship =================
            # contiguous view of the DRAM table: row r = p*nblk + b
            tsv = tslice[:, :].rearrange("(p b) e -> p (b e)", p=128)
            tsb = tslice[:, :].rearrange("(o p b) e -> o p (b e)", o=1,
                                         p=128)
            nch = 4
            C_BOUNDS = [nblk * i // nch for i in range(nch + 1)]

            def table_ship(table_sb, tf, lo, hi):
                """DMA table blocks [lo,hi) to DRAM + mock-AllGather them."""
                nc.sync.dma_start(
                    tsv[:, lo * ROW:hi * ROW],
                    table_sb[:, lo * ROW:hi * ROW])
                if hi == nblk:
                    # pad row: overwrite alpha_src of row npc-1 (dummy)
                    nc.sync.dma_start(
                        tslice[npc - 1:npc, d_hid:d_hid + h], padrow[:])
                if mock_cc:
                    # cost-model stand-in: move the same bytes the AllGather
                    # would receive (cores-1 slices in + 1 local copy), as a
                    # broadcast-read DMA per chunk
                    tfv = tf[:, :].rearrange("(c p b) e -> c p (b e)",
                                             c=cores, p=128)
                    nc.sync.dma_start(
                        tfv[:, :, lo * ROW:hi * ROW],
                        tsb[:, :, lo * ROW:hi * ROW].broadcast_to(
                            [cores, 128, (hi - lo) * ROW]))

            def table_block(lhsT, Wc_sb, table_sb, b):
                ps = psA.tile([128, DCAT], mybir.dt.float32)
                nc.tensor.matmul(ps[:], lhsT, Wc_sb[:, :], start=True,
                                 stop=True)
                # f32 PSUM -> bf16 table tile on the ACT engine
                nc.scalar.activation(
                    table_sb[:, b * ROW:b * ROW + DCAT],
                    ps[:, 0:DCAT], AF.Copy)

            def table_build(src_lhsT, Wc_sb, table_sb, tf):
                """src_lhsT(b) -> lhsT AP [k, 128] for block b."""
                for ci in range(nch):
                    lo, hi = C_BOUNDS[ci], C_BOUNDS[ci + 1]
                    for b in range(lo, hi):
                        table_block(src_lhsT(b), Wc_sb, table_sb, b)
                    table_ship(table_sb, tf, lo, hi)
                if not mock_cc:
                    nc.gpsimd.collective_compute(
                        "AllGather", mybir.AluOpType.bypass,
                        replica_groups=[list(range(cores))],
                        ins=[tslice[:, :]], outs=[tf[:, :]])

            # ================= helper: edge phase =================
            def edge_phase(tf, table_sb, post):
                """post(b, ps) consumes psum [128, ZCOL] for block b.

                Gathers are merged across groups of consecutive blocks to
                amortize the ~1us SWDGE fixed cost per dma_gather; edge math
                runs group-wide where it is block-agnostic.
                """
                for (g0, g1) in plan["groups"]:
                    KLg = int(off_lo[g1] - off_lo[g0]) // 128
                    KHg = int(off_hi[g1] - off_hi[g0]) // 128
                    Kg = KLg + KHg
                    if Kg == 0:
                        for b in range(g0, g1):
                            post(b, None)
                        continue
                    G = gath.tile([128, GK_MAX * 128], TDT, tag="G")
                    if KLg:
                        nc.gpsimd.dma_gather(
                            G[:, :KLg * 128].rearrange("p (k e) -> p k e",
                                                       e=128),
                            tf[0:half, :],
                            ixlo_sb[:, off_lo[g0] // 16:off_lo[g1] // 16],
                            128 * KLg, 128 * KLg, ROW,
                            single_packet=False)
                    if KHg:
                        nc.gpsimd.dma_gather(
                            G[:, KLg * 128:Kg * 128].rearrange(
                                "p (k e) -> p k e", e=128),
                            tf[half:ntot, :],
                            ixhi_sb[:, off_hi[g0] // 16:off_hi[g1] // 16],
                            128 * KHg, 128 * KHg, ROW,
                            single_packet=False)
                    Gv = G[:, :Kg * 128].rearrange("p (k e) -> p k e", e=128)

                    def blk_ranges(b):
                        lo = ((off_lo[b] - off_lo[g0]) // 128,
                              (off_lo[b + 1] - off_lo[g0]) // 128)
                        hi = (KLg + (off_hi[b] - off_hi[g0]) // 128,
                              KLg + (off_hi[b + 1] - off_hi[g0]) // 128)
                        return [r for r in (lo, hi) if r[1] > r[0]]

                    # e = alpha_src + alpha_dst (per block: ad varies); the
                    # local table tile holds alpha_dst at cols ZCOL:DCAT
                    for b in range(g0, g1):
                        adc = table_sb[:, b * ROW + ZCOL:b * ROW + DCAT]
                        for (c0, c1) in blk_ranges(b):
                            kk = int(c1 - c0)
                            nc.vector.tensor_add(
                                Gv[:, c0:c1, DCAT:DCAT + h],
                                Gv[:, c0:c1, d_hid:d_hid + h],
                                adc.rearrange("p (o j) -> p o j",
                                              o=1).broadcast_to([128, kk, h]))
                    # exp(leakyrelu(e)) == exp(max(e, 0.2e)) -- group-wide
                    ev = Gv[:, :, DCAT:DCAT + h]
                    nc.vector.scalar_tensor_tensor(
                        ev, ev, NEG_SLOPE, ev, op0=ALU.mult, op1=ALU.max)
                    # 12-wide expanded weights (packed) for the message mul
                    SE = sexp.tile([128, GK_MAX * d_hid], TDT, tag="SE")
                    SEv = SE[:, :Kg * d_hid].rearrange(
                        "p (k j d) -> p k j d", j=h, d=dh)
                    nc.scalar.activation(
                        SEv,
                        ev.rearrange("p k (j o) -> p k j o",
                                     o=1).broadcast_to([128, Kg, h, dh]),
                        AF.Exp)
                    # 8-wide weights into cols d_hid:ZCOL (accumulated as the
                    # softmax denominator); strided copy of lane 0 of each
                    # head from SE.  Overwrites gathered alpha_src, which the
                    # e-adds above already consumed.
                    nc.vector.tensor_copy(
                        Gv[:, :, d_hid:d_hid + h].rearrange(
                            "p k (j o) -> p k j o", o=1),
                        SE[:, :Kg * d_hid].rearrange(
                            "p (k j d) -> p k j d", j=h, d=dh)[:, :, :, 0:1])
                    # fully-packed bf16 multiply (2x DVE mode)
                    nc.vector.tensor_mul(
                        Gv[:, :, 0:d_hid], Gv[:, :, 0:d_hid],
                        SE[:, :Kg * d_hid].rearrange("p (k f) -> p k f",
                                                     f=d_hid))
                    # per-block accumulate [msg | s_exp] via identity matmul
                    for b in range(g0, g1):
                        chunks = [c for (c0, c1) in blk_ranges(b)
                                  for c in range(c0, c1)]
                        if not chunks:
                            post(b, None)
                            continue
                        ps = psB.tile([128, ZCOL], mybir.dt.float32)
                        for i, c in enumerate(chunks):
                            nc.tensor.matmul(
                                ps[:], idt_sb[:, :],
                                G[:, c * 128:c * 128 + ZCOL],
                                start=(i == 0), stop=(i == len(chunks) - 1))
                        post(b, ps)

            # ======= helper: output MLP + log_softmax (batched blocks) =====
            # Per block: transpose + matmul into a shared multi-block PSUM
            # tile.  Every OUT_B blocks one batched epilogue computes
            # out = t0 - ln(sum(exp(t0))) over [128, OUT_B*n_cls] at once
            # (the max-shift is skipped: logits are O(1)).
            OUT_B = 7
            ostate = {"po": None, "b0": 0, "cnt": 0}

            def out_flush():
                nb, po = ostate["cnt"], ostate["po"]
                if not nb:
                    return
                b0 = ostate["b0"]
                w = nb * n_cls
                # biased logits straight into out_sb; the -ln(z) shift is
                # applied once, globally, after phase D (keeps ACT on the Exp
                # table the whole phase)
                ov = out_sb[:, b0 * n_cls:b0 * n_cls + w]
                nc.vector.tensor_add(
                    ov.rearrange("p (b c) -> p b c", c=n_cls),
                    po[:, 0:w].rearrange("p (b c) -> p b c", c=n_cls),
                    bo_sb[:, :].rearrange("p (o c) -> p o c",
                                          o=1).broadcast_to([128, nb, n_cls]))
                ex = stage.tile([128, OUT_B * n_cls], mybir.dt.float32,
                                tag="ex")
                nc.scalar.activation(ex[:, 0:w], ov, AF.Exp)
                nc.vector.reduce_sum(
                    s_all[:, b0:b0 + nb],
                    ex[:, 0:w].rearrange("p (b c) -> p b c", c=n_cls),
                    axis=mybir.AxisListType.X)
                ostate["po"] = None
                ostate["cnt"] = 0

            def out_finish():
                ls = small.tile([128, nblk], mybir.dt.float32, tag="ls")
                nc.scalar.activation(ls[:], s_all[:, :], AF.Ln)
                nc.vector.tensor_sub(
                    out_sb[:, :].rearrange("p (b c) -> p b c", c=n_cls),
                    out_sb[:, :].rearrange("p (b c) -> p b c", c=n_cls),
                    ls[:, :].rearrange("p (b o) -> p b o",
                                       o=1).broadcast_to([128, nblk, n_cls]))

            def transpose_h(hv):
                """[128, d_hid] bf16 -> [d_hid, 128] bf16 SBUF (copy on ACT,
                which is off the DVE critical path)."""
                pst = psT.tile([d_hid, 128], TDT, tag="ptr")
                nc.tensor.transpose(pst[:], hv, idt_sb[:, :])
                hT = stage.tile([d_hid, 128], TDT, tag="hT")
                nc.scalar.activation(hT[:], pst[:], AF.Copy)
                return hT

            def out_block(b, hv):
                """hv: [128, d_hid] bf16 SBUF view of layer-2 activations."""
                hT = transpose_h(hv)
                if ostate["po"] is None:
                    po7 = psO.tile([128, OUT_B * n_cls], mybir.dt.float32,
                                   tag="po", name="po7")
                    ostate["po"] = po7
                    ostate["b0"] = b
                i = ostate["cnt"]
                nc.tensor.matmul(
                    ostate["po"][:, i * n_cls:(i + 1) * n_cls],
                    hT[:, :], Wo_sb[:, :], start=True, stop=True)
                ostate["cnt"] = i + 1
                if ostate["cnt"] == OUT_B:
                    out_flush()

            bailed = False

            # ================= phase A: table 0 =================
            for _rep in range(repeat):
              tfull0, tfull1 = tfulls[_rep]
              table_build(
                  lambda b: xT_sb[:, b * 128:(b + 1) * 128],
                  W0_sb, table0_sb, tfull0)

              if stop_after == "A":
                  bailed = True

              # ========== phase B: layer-0 edges + fused table-1 build =====
              # the layer-1 table pipeline (transpose/matmul/convert/ship)
              # for block b is emitted right in post0(b) so it streams
              # through phase B instead of bunching at the B->C boundary
              def post0(b, ps):
                  hv = h1_sb[:, b * d_hid:(b + 1) * d_hid]
                  if ps is None:
                      nc.vector.tensor_copy(hv, b0_sb[:, :])
                  else:
                      z = small.tile([128, h], mybir.dt.float32, tag="z")
                      nc.vector.tensor_scalar_add(z[:], ps[:, d_hid:ZCOL],
                                                  1e-16)
                      iz = small.tile([128, h], mybir.dt.float32, tag="iz")
                      nc.vector.reciprocal(iz[:], z[:])
                      izb = iz[:, :].rearrange("p (j o) -> p j o",
                                               o=1).broadcast_to([128, h, dh])
                      hv3 = hv.rearrange("p (j d) -> p j d", d=dh)
                      nc.vector.tensor_mul(hv3, ps[:, 0:d_hid].rearrange(
                          "p (j d) -> p j d", d=dh), izb)
                      nc.vector.tensor_add(hv, hv, b0_sb[:, :])
                  table_block(transpose_h(hv)[:, :], W1_sb, table1_sb, b)
                  if b + 1 in C_BOUNDS:
                      ci = C_BOUNDS.index(b + 1)
                      table_ship(table1_sb, tfull1, C_BOUNDS[ci - 1], b + 1)

              if not bailed:
                  edge_phase(tfull0, table0_sb, post0)
                  if not mock_cc:
                      nc.gpsimd.collective_compute(
                          "AllGather", mybir.AluOpType.bypass,
                          replica_groups=[list(range(cores))],
                          ins=[tslice[:, :]], outs=[tfull1[:, :]])
              if stop_after == "B":
                  bailed = True

              if stop_after == "C":
                  bailed = True

              # ======== phase D: layer-1 edges + fused output MLP ========
              # out_block emission lags post1 by a few blocks so the next
              # group's gather-releasing accumulate matmuls get PE priority
              pend = []

              def post1(b, ps):
                  hv = h2_sb[:, b * d_hid:(b + 1) * d_hid]
                  if ps is None:
                      t = small.tile([128, d_hid], mybir.dt.float32, tag="t1")
                      nc.vector.tensor_copy(t[:], b1_sb[:, :])
                      nc.vector.tensor_scalar_max(hv, t[:], 0.0)
                  else:
                      z = small.tile([128, h], mybir.dt.float32, tag="z")
                      nc.vector.tensor_scalar_add(z[:], ps[:, d_hid:ZCOL],
                                                  1e-16)
                      iz = small.tile([128, h], mybir.dt.float32, tag="iz")
                      nc.vector.reciprocal(iz[:], z[:])
                      izb = iz[:, :].rearrange("p (j o) -> p j o",
                                               o=1).broadcast_to([128, h, dh])
                      t = small.tile([128, d_hid], mybir.dt.float32, tag="t1")
                      t3 = t[:, :].rearrange("p (j d) -> p j d", d=dh)
                      nc.vector.tensor_mul(t3, ps[:, 0:d_hid].rearrange(
                          "p (j d) -> p j d", d=dh), izb)
                      nc.vector.tensor_add(t[:], t[:], b1_sb[:, :])
                      nc.vector.tensor_scalar_max(hv, t[:], 0.0)  # ReLU
                  pend.append((b, hv))
                  if len(pend) > 3:
                      out_block(*pend.pop(0))

              if not bailed:
                  edge_phase(tfull1, table1_sb, post1)
                  while pend:
                      out_block(*pend.pop(0))
                  out_flush()
                  out_finish()

            if bailed:
                nc.vector.memset(out_sb[:, :], 0.0)
            # one contiguous DMA: out row r = p*nblk + b  <=>  [p, (b c)]
            nc.sync.dma_start(
                out_d[:, :].rearrange("(p b) c -> p (b c)", p=128),
                out_sb[:, :])

    nc.compile()
    return nc


# ---------------------------------------------------------------- inputs
def make_in_maps(plan, inputs, f_in=F_IN, d_hid=D_HID, h=H, n_cls=N_CLS,
                 bf16=True):
    import ml_dtypes

    x = np.asarray(inputs["x"], dtype=np.float32)
    W0 = np.asarray(inputs["W0"], dtype=np.float32)
    W1 = np.asarray(inputs["W1"], dtype=np.float32)
    Wout = np.asarray(inputs["Wout"], dtype=np.float32)
    as0 = np.asarray(inputs["as0"], dtype=np.float32)
    ad0 = np.asarray(inputs["ad0"], dtype=np.float32)
    as1 = np.asarray(inputs["as1"], dtype=np.float32)
    ad1 = np.asarray(inputs["ad1"], dtype=np.float32)
    b0 = np.asarray(inputs["b0"], dtype=np.float32)
    b1 = np.asarray(inputs["b1"], dtype=np.float32)
    bout = np.asarray(inputs["bout"], dtype=np.float32)

    dh = d_hid // h
    npc, cores = plan["npc"], plan["cores"]
    old_of_slot = plan["old_of_slot"]

    def blockdiag(a):  # [h, dh] -> [d_hid, h]
        m = np.zeros((d_hid, h), dtype=np.float32)
        for j in range(h):
            m[j * dh:(j + 1) * dh, j] = a[j]
        return m

    W0cat = np.concatenate(
        [W0, W0 @ blockdiag(as0), W0 @ blockdiag(ad0)], axis=1)
    W1cat = np.concatenate(
        [W1, W1 @ blockdiag(as1), W1 @ blockdiag(ad1)], axis=1)

    tdt = ml_dtypes.bfloat16
    ident = np.eye(128, dtype=tdt)
    b0b = np.ascontiguousarray(np.broadcast_to(b0, (128, d_hid)))
    b1b = np.ascontiguousarray(np.broadcast_to(b1, (128, d_hid)))
    boutb = np.ascontiguousarray(np.broadcast_to(bout, (128, n_cls)))

    x_ext = np.zeros((npc * cores, f_in), dtype=np.float32)
    real = old_of_slot >= 0
    x_ext[real] = x[old_of_slot[real]]

    in_maps = []
    for c in range(cores):
        xs = x_ext[c * npc:(c + 1) * npc]
        m = dict(
            xT=np.ascontiguousarray(xs.T).astype(tdt),
            W0cat=W0cat.astype(tdt), W1cat=W1cat.astype(tdt),
            Wout=Wout.astype(tdt),
            b0b=b0b, b1b=b1b, boutb=boutb,
            identt=ident,
            idx_lo=np.ascontiguousarray(plan["idx_lo"][c])
            if plan["S_lo"] else np.zeros((128, 1), np.int16),
            idx_hi=np.ascontiguousarray(plan["idx_hi"][c])
            if plan["S_hi"] else np.zeros((128, 1), np.int16),
        )
        in_maps.append(m)
    return in_maps


def assemble_output(plan, results, n_cls=N_CLS):
    outs = np.concatenate([r["out"] for r in results], axis=0)
    return np.ascontiguousarray(outs[plan["row_of_old"]], dtype=np.float32)


# ---------------------------------------------------------------- entry
def kernel(**inputs):
    from concourse.bass_utils import run_bass_kernel_spmd

    edge_src = np.asarray(inputs["edge_src"]).astype(np.int64)
    edge_dst = np.asarray(inputs["edge_dst"]).astype(np.int64)

    bf16 = True
    key = (edge_src.tobytes(), edge_dst.tobytes(), bf16)
    kh = hash(key)
    if kh not in _CACHE:
        plan = make_plan(edge_src, edge_dst)
        nc = build_program(plan, bf16=bf16)
        _CACHE[kh] = (plan, nc)
    plan, nc = _CACHE[kh]

    in_maps = make_in_maps(plan, inputs, bf16=bf16)
    res = run_bass_kernel_spmd(nc, in_maps,
                               core_ids=list(range(plan["cores"])))
    return assemble_output(plan, res.results)


# revision 37
# speedup vs baseline: 1.5530x; 1.0269x over previous
"""GAT (2-layer graph attention network + output MLP) on 8 Trainium2 NeuronCores.

Strategy ("diagonal scheduling", v2):
  - The Bass program is built per-invocation, so the graph structure is a
    compile-time constant.  Nodes are assigned to cores balancing total
    in-degree, and within each core nodes are grouped into 128-node blocks
    sorted by (deg_lo, deg_hi) so that all nodes in a block have nearly equal
    in-degree from each half of the node space.
  - Edges of a block are laid out in "chunks" of 128 slots: slot (c, p) holds
    the c-th in-edge of the node on partition p.  A chunk therefore has at
    most one edge per destination, which turns the segment-softmax scatter
    into a plain PSUM accumulation with a constant identity stationary matrix
    (no masks, no segment ops).
  - Per layer, each core computes the feature/attention table rows for its own
    nodes ([h | alpha_src | alpha_dst] per node), all-gathers the full table to
    DRAM, and then gathers per-edge rows with dma_gather (int16 indices force
    a lo/hi table split at NTOT/2).
  - Table rows are numbered partition-major (row = p*nblk + b) so the whole
    per-core table slice is written to DRAM with ONE contiguous DMA straight
    from the SBUF-resident table tile (which also serves as the local
    alpha_dst source).  The PSUM->SBUF bf16 convert runs on the otherwise-idle
    Activation engine.
  - softmax: exp(leakyrelu(e)) == exp(max(e, 0.2e)) -- one DVE
    scalar_tensor_tensor (mult/max), then two ACT exps: an 8-wide one (the
    per-edge weights accumulated for the softmax denominator) and a 12-wide
    broadcast-EXPANDED one so the message multiply is a fully-packed bf16
    TensorTensor (2x DVE mode).  The 1/z normalization folds in after
    aggregation (exact same math as the reference; max-subtraction is skipped
    since logits are O(1)).
  - The output MLP + log_softmax is folded into layer-1's per-block epilogue
    and accumulated in SBUF; one contiguous DMA writes the result at the end.

kernel(**inputs) -> np.ndarray  takes full inputs, returns the full output.
"""

import numpy as np

# ---------------------------------------------------------------- constants
N, E, F_IN, D_HID, H, N_CLS = 50000, 800000, 128, 96, 8, 40
DH = D_HID // H  # 12
NEG_SLOPE = 0.2
CORES = 8
BLK = 128
PAD_AS = -10000.0  # alpha_src for pad rows: exp(leaky(PAD_AS+ad)) == 0.0
GK_MAX = 64  # max chunks per merged-gather group

_CACHE = {}


# ---------------------------------------------------------------- planning
def make_plan(edge_src, edge_dst, n=N, cores=CORES, blk=BLK):
    """Pure graph-structure planning (numpy only).

    Returns a dict with the node permutations, per-block common chunk counts
    and the per-core wrapped int16 gather-index arrays.  Node ids come in two
    numberings: "slot-major" (slot s = b*128 + p; used for the xT input
    layout) and "row-major" (table row r = p*nblk + b; used for the DRAM
    table, the gather indices and the output layout).
    """
    edge_src = np.asarray(edge_src, dtype=np.int64)
    edge_dst = np.asarray(edge_dst, dtype=np.int64)
    e = len(edge_src)

    deg = np.bincount(edge_dst, minlength=n)

    # nodes per core, including dummies; one dummy pinned last on every core
    npc = -(-(n + cores) // (cores * blk)) * blk  # round up to block multiple
    ntot = cores * npc
    half = ntot // 2
    nblk = npc // blk

    # --- assign real nodes to cores balancing total degree (snake deal) ---
    order = np.argsort(-deg, kind="stable")  # real nodes by degree desc
    core_of = np.empty(n, dtype=np.int64)
    r = np.arange(n)
    rnd = r // cores
    pos = r % cores
    fwd = (rnd % 2) == 0
    lane = np.where(fwd, pos, cores - 1 - pos)
    core_of[order] = lane

    # lo set = cores 0..cores/2-1
    is_lo_node = core_of < (cores // 2)
    src_is_lo = is_lo_node[edge_src]
    d_lo = np.bincount(edge_dst[src_is_lo], minlength=n)
    d_hi = deg - d_lo

    # --- per-half global ordering, dealt round-robin to the half's cores ---
    # Sorting each half globally by (d_lo desc, d_hi snake) and dealing node
    # at sorted position g to core g%hc, slot g//hc keeps every core's block
    # profile an interleaved sample of the same distribution, so the common
    # (cross-core max) chunk counts stay tight.  Dummies sort last, which
    # pins one dummy at the final slot of every core (used as the pad row).
    slot_of_old = np.empty(n, dtype=np.int64)  # slot-major global id
    old_of_slot = np.full(ntot, -1, dtype=np.int64)
    hc = cores // 2
    for side in (0, 1):
        mine = np.where(is_lo_node == (side == 0))[0]
        dl, dhh = d_lo[mine], d_hi[mine]
        run_parity = (dl.max() - dl) % 2  # alternate d_hi dir per d_lo run
        key_hi = np.where(run_parity == 0, -dhh, dhh)
        srt = mine[np.lexsort((key_hi, -dl))]
        assert len(srt) <= hc * npc - hc, (len(srt), npc)
        g = np.arange(len(srt))
        core = side * hc + g % hc
        slot = g // hc
        newids = core * npc + slot
        slot_of_old[srt] = newids
        old_of_slot[newids] = srt

    # slot-major local id s -> row-major local id r = (s%128)*nblk + s//128
    s_loc = np.arange(npc)
    perm_row = (s_loc % blk) * nblk + s_loc // blk  # row id of slot s
    row_of_old = (slot_of_old // npc) * npc + perm_row[slot_of_old % npc]

    # --- per (core, block) lo/hi chunk counts -> common across cores ---
    d_lo_s = np.zeros(ntot, dtype=np.int64)
    d_hi_s = np.zeros(ntot, dtype=np.int64)
    real = old_of_slot >= 0
    d_lo_s[real] = d_lo[old_of_slot[real]]
    d_hi_s[real] = d_hi[old_of_slot[real]]
    kl_cb = d_lo_s.reshape(cores, nblk, blk).max(axis=2)
    kh_cb = d_hi_s.reshape(cores, nblk, blk).max(axis=2)
    K_lo = kl_cb.max(axis=0)  # [nblk] common
    K_hi = kh_cb.max(axis=0)

    off_lo = np.concatenate([[0], np.cumsum(K_lo * blk)])  # slot offsets
    off_hi = np.concatenate([[0], np.cumsum(K_hi * blk)])
    S_lo = int(off_lo[-1])
    S_hi = int(off_hi[-1])

    # pad row: slot npc-1 (the pinned dummy) of core cores/2-1 maps to row
    # npc-1 (p=127, b=nblk-1), i.e. global row half-1 -- same value as the
    # slot-major scheme.
    pad_lo = half - 1
    pad_hi = half - 1  # (value in hi-table local coords: ntot-1-half)

    # --- slot filling ---
    dst_s = slot_of_old[edge_dst]
    src_row = row_of_old[edge_src]
    is_lo = src_row < half

    # rank of each edge within its (dst, class) group
    grp = dst_s * 2 + (~is_lo)
    srt = np.argsort(grp, kind="stable")
    grp_s = grp[srt]
    starts = np.concatenate([[0], np.where(np.diff(grp_s) != 0)[0] + 1])
    group_start = np.zeros(len(grp_s), dtype=np.int64)
    group_start[starts] = starts
    group_start = np.maximum.accumulate(group_start)
    rank_s = np.arange(e) - group_start
    rank = np.empty(e, dtype=np.int64)
    rank[srt] = rank_s

    core_e = dst_s // npc
    blk_e = (dst_s % npc) // blk
    p_e = dst_s % blk

    slots_lo = np.full((cores, S_lo), pad_lo, dtype=np.int16)
    slots_hi = np.full((cores, S_hi), pad_hi, dtype=np.int16)

    lo_m = is_lo
    pos_lo = off_lo[blk_e[lo_m]] + rank[lo_m] * blk + p_e[lo_m]
    slots_lo[core_e[lo_m], pos_lo] = src_row[lo_m].astype(np.int16)
    hi_m = ~is_lo
    pos_hi = off_hi[blk_e[hi_m]] + rank[hi_m] * blk + p_e[hi_m]
    slots_hi[core_e[hi_m], pos_hi] = (src_row[hi_m] - half).astype(np.int16)

    # wrap for dma_gather: element i -> [i%16, i//16], tiled to 128 partitions
    def wrap(a):
        s = a.shape[1]
        if s == 0:
            return np.zeros((cores, 128, 0), dtype=np.int16)
        w = a.reshape(cores, s // 16, 16).transpose(0, 2, 1)  # [cores,16,S/16]
        return np.ascontiguousarray(np.tile(w, (1, 8, 1)))

    # --- group consecutive blocks for merged gathers ---
    # the last TAIL_SINGLES blocks get singleton groups so the post-gather
    # compute drain at the end of each edge phase is short
    TAIL_SINGLES = 2
    groups = []  # list of (b0, b1)  [b0, b1) blocks
    b0 = 0
    ktot = K_lo + K_hi
    while b0 < nblk:
        if nblk - b0 <= TAIL_SINGLES:
            groups.append((b0, b0 + 1))
            b0 += 1
            continue
        b1 = b0 + 1
        s = int(ktot[b0])
        while (b1 < nblk and s + int(ktot[b1]) <= GK_MAX
               and b1 - b0 < 4 and nblk - b1 > TAIL_SINGLES):
            s += int(ktot[b1])
            b1 += 1
        groups.append((b0, b1))
        b0 = b1

    return dict(
        n=n, e=e, cores=cores, npc=npc, ntot=ntot, half=half, nblk=nblk,
        slot_of_old=slot_of_old, old_of_slot=old_of_slot,
        row_of_old=row_of_old,
        K_lo=K_lo.astype(np.int64), K_hi=K_hi.astype(np.int64),
        off_lo=off_lo, off_hi=off_hi, S_lo=S_lo, S_hi=S_hi,
        idx_lo=wrap(slots_lo), idx_hi=wrap(slots_hi),
        groups=groups,
        util=float(e) / max(1.0, float((S_lo + S_hi) * cores)),
    )


# ---------------------------------------------------------------- program
def build_program(plan, f_in=F_IN, d_hid=D_HID, h=H, n_cls=N_CLS, bf16=True,
                  stop_after=None, repeat=1, mock_cc=False, acc="pe"):
    import concourse.bacc as bacc
    import concourse.mybir as mybir
    from concourse import tile

    dt = mybir.dt
    f32 = dt.float32
    TDT = dt.bfloat16
    AF = mybir.ActivationFunctionType
    ALU = mybir.AluOpType
    dh = d_hid // h
    npc, nblk, half = plan["npc"], plan["nblk"], plan["half"]
    K_lo, K_hi = plan["K_lo"], plan["K_hi"]
    off_lo, off_hi = plan["off_lo"], plan["off_hi"]
    S_lo, S_hi = plan["S_lo"], plan["S_hi"]
    cores = plan["cores"]
    ntot = plan["ntot"]
    ROW = 128  # table row, elements (256B in bf16 -- dma_gather granule)
    DCAT = d_hid + 2 * h  # 112: [h | alpha_src | alpha_dst]
    ZCOL = d_hid + h  # 104: columns accumulated in PSUM (msg | s_exp)

    nc = bacc.Bacc("TRN2", target_bir_lowering=False, debug=False,
                   num_devices=cores)

    # ---- I/O ----
    xT = nc.dram_tensor("xT", [f_in, npc], TDT, kind="ExternalInput")
    W0cat = nc.dram_tensor("W0cat", [f_in, DCAT], TDT, kind="ExternalInput")
    W1cat = nc.dram_tensor("W1cat", [d_hid, DCAT], TDT, kind="ExternalInput")
    Wout = nc.dram_tensor("Wout", [d_hid, n_cls], TDT, kind="ExternalInput")
    b0b = nc.dram_tensor("b0b", [128, d_hid], f32, kind="ExternalInput")
    b1b = nc.dram_tensor("b1b", [128, d_hid], f32, kind="ExternalInput")
    boutb = nc.dram_tensor("boutb", [128, n_cls], f32, kind="ExternalInput")
    identt = nc.dram_tensor("identt", [128, 128], TDT, kind="ExternalInput")
    idx_lo_d = nc.dram_tensor("idx_lo", [128, max(S_lo // 16, 1)], dt.int16,
                              kind="ExternalInput")
    idx_hi_d = nc.dram_tensor("idx_hi", [128, max(S_hi // 16, 1)], dt.int16,
                              kind="ExternalInput")
    out_d = nc.dram_tensor("out", [npc, n_cls], f32, kind="ExternalOutput")

    with tile.TileContext(nc) as tc:
        with (
            tc.tile_pool(name="dram", bufs=1, space="DRAM") as dramp,
            tc.tile_pool(name="persist", bufs=1) as pers,
            tc.tile_pool(name="gath", bufs=4) as gath,
            tc.tile_pool(name="sexp", bufs=3) as sexp,
            tc.tile_pool(name="stage", bufs=4) as stage,
            tc.tile_pool(name="small", bufs=4) as small,
            tc.tile_pool(name="psA", bufs=2, space="PSUM") as psA,
            tc.tile_pool(name="psB", bufs=2, space="PSUM") as psB,
            tc.tile_pool(name="psT", bufs=2, space="PSUM") as psT,
            tc.tile_pool(name="psO", bufs=1, space="PSUM") as psO,
        ):
            # ---- DRAM scratch ----
            tslice = dramp.tile([npc, ROW], TDT)
            aspace = "Local" if mock_cc else "Shared"
            tfulls = [
                (dramp.tile([ntot, ROW], TDT, addr_space=aspace,
                            name=f"tf0_{r}", tag=f"tf0_{r}"),
                 dramp.tile([ntot, ROW], TDT, addr_space=aspace,
                            name=f"tf1_{r}", tag=f"tf1_{r}"))
                for r in range(repeat)
            ]

            # ---- persistent SBUF ----
            xT_sb = pers.tile([f_in, npc], TDT)
            nc.sync.dma_start(xT_sb[:], xT[:, :])
            W0_sb = pers.tile([f_in, DCAT], TDT)
            nc.sync.dma_start(W0_sb[:], W0cat[:, :])
            W1_sb = pers.tile([d_hid, DCAT], TDT)
            nc.sync.dma_start(W1_sb[:], W1cat[:, :])
            Wo_sb = pers.tile([d_hid, n_cls], TDT)
            nc.sync.dma_start(Wo_sb[:], Wout[:, :])
            b0_sb = pers.tile([128, d_hid], f32)
            nc.sync.dma_start(b0_sb[:], b0b[:, :])
            b1_sb = pers.tile([128, d_hid], f32)
            nc.sync.dma_start(b1_sb[:], b1b[:, :])
            bo_sb = pers.tile([128, n_cls], f32)
            nc.sync.dma_start(bo_sb[:], boutb[:, :])
            idt_sb = pers.tile([128, 128], TDT)
            nc.sync.dma_start(idt_sb[:], identt[:, :])
            if S_lo:
                ixlo_sb = pers.tile([128, S_lo // 16], dt.int16)
                nc.sync.dma_start(ixlo_sb[:], idx_lo_d[:, :])
            if S_hi:
                ixhi_sb = pers.tile([128, S_hi // 16], dt.int16)
                nc.sync.dma_start(ixhi_sb[:], idx_hi_d[:, :])
            table0_sb = pers.tile([128, nblk * ROW], TDT)
            table1_sb = pers.tile([128, nblk * ROW], TDT)
            h1_sb = pers.tile([128, nblk * d_hid], TDT)
            h2_sb = pers.tile([128, nblk * d_hid], TDT)
            out_sb = pers.tile([128, nblk * n_cls], f32)
            s_all = pers.tile([128, nblk], f32)
            padrow = pers.tile([1, h], TDT)
            nc.vector.memset(padrow[:], PAD_AS)
            # pad cols (DCAT:ROW) are shipped by the contiguous table DMA;
            # zero them once so no uninitialized bytes flow
            for tbl in (table0_sb, table1_sb):
                nc.vector.memset(
                    tbl[:, :].rearrange("p (b e) -> p b e",
                                        e=ROW)[:, :, DCAT:ROW], 0.0)

            # ================= helpers: table build/ship =================
            # contiguous view of the DRAM table: row r = p*nblk + b
            tsv = tslice[:, :].rearrange("(p b) e -> p (b e)", p=128)
            tsb = tslice[:, :].rearrange("(o p b) e -> o p (b e)", o=1,
                                         p=128)
            # uneven chunks: small final chunk so the last ship (which gates
            # the next phase's gathers) trails the final blocks closely
            C_BOUNDS = sorted({round(nblk * f) for f in
                               (0, 0.27, 0.54, 0.8, 0.94, 1.0)})
            nch = len(C_BOUNDS) - 1

            def table_ship(table_sb, tf, lo, hi):
                """DMA table blocks [lo,hi) to DRAM + mock-AllGather them."""
                nc.sync.dma_start(
                    tsv[:, lo * ROW:hi * ROW],
                    table_sb[:, lo * ROW:hi * ROW])
                if hi == nblk:
                    # pad row: overwrite alpha_src of row npc-1 (dummy)
                    nc.sync.dma_start(
                        tslice[npc - 1:npc, d_hid:d_hid + h], padrow[:])
                if mock_cc:
                    # cost-model stand-in: move the same bytes the AllGather
                    # would receive (cores-1 slices in + 1 local copy), as a
                    # broadcast-read DMA per chunk
                    tfv = tf[:, :].rearrange("(c p b) e -> c p (b e)",
                                             c=cores, p=128)
                    nc.sync.dma_start(
                        tfv[:, :, lo * ROW:hi * ROW],
                        tsb[:, :, lo * ROW:hi * ROW].broadcast_to(
                            [cores, 128, (hi - lo) * ROW]))

            def table_block(lhsT, Wc_sb, table_sb, b):
                ps = psA.tile([128, DCAT], mybir.dt.float32)
                nc.tensor.matmul(ps[:], lhsT, Wc_sb[:, :], start=True,
                                 stop=True)
                # f32 PSUM -> bf16 table tile on the ACT engine
                nc.scalar.activation(
                    table_sb[:, b * ROW:b * ROW + DCAT],
                    ps[:, 0:DCAT], AF.Copy)

            def table_build(src_lhsT, Wc_sb, table_sb, tf):
                """src_lhsT(b) -> lhsT AP [k, 128] for block b."""
                for ci in range(nch):
                    lo, hi = C_BOUNDS[ci], C_BOUNDS[ci + 1]
                    for b in range(lo, hi):
                        table_block(src_lhsT(b), Wc_sb, table_sb, b)
                    table_ship(table_sb, tf, lo, hi)
                if not mock_cc:
                    nc.gpsimd.collective_compute(
                        "AllGather", mybir.AluOpType.bypass,
                        replica_groups=[list(range(cores))],
                        ins=[tslice[:, :]], outs=[tf[:, :]])

            GBMAX = max(g1 - g0 for (g0, g1) in plan["groups"])

            # ================= helper: edge phase =================
            def edge_phase(tf, table_sb, post_group):
                """post_group(b0, gb, psG) consumes psum [128, gb*ZCOL]
                holding the accumulated [msg | s_exp] of blocks b0..b0+gb.

                Gathers are merged across groups of consecutive blocks to
                amortize the ~1us SWDGE fixed cost per dma_gather; edge math
                runs range-wide where it is block-agnostic.
                """
                for (g0, g1) in plan["groups"]:
                    KLg = int(off_lo[g1] - off_lo[g0]) // 128
                    KHg = int(off_hi[g1] - off_hi[g0]) // 128
                    Kg = KLg + KHg
                    gb = g1 - g0
                    if Kg == 0:
                        post_group(g0, gb, None)
                        continue
                    G = gath.tile([128, GK_MAX * 128], TDT, tag="G")
                    if KLg:
                        nc.gpsimd.dma_gather(
                            G[:, :KLg * 128].rearrange("p (k e) -> p k e",
                                                       e=128),
                            tf[0:half, :],
                            ixlo_sb[:, off_lo[g0] // 16:off_lo[g1] // 16],
                            128 * KLg, 128 * KLg, ROW,
                            single_packet=False)
                    if KHg:
                        nc.gpsimd.dma_gather(
                            G[:, KLg * 128:Kg * 128].rearrange(
                                "p (k e) -> p k e", e=128),
                            tf[half:ntot, :],
                            ixhi_sb[:, off_hi[g0] // 16:off_hi[g1] // 16],
                            128 * KHg, 128 * KHg, ROW,
                            single_packet=False)
                    Gv = G[:, :Kg * 128].rearrange("p (k e) -> p k e", e=128)

                    def blk_ranges(b):
                        lo = ((off_lo[b] - off_lo[g0]) // 128,
                              (off_lo[b + 1] - off_lo[g0]) // 128)
                        hi = (KLg + (off_hi[b] - off_hi[g0]) // 128,
                              KLg + (off_hi[b + 1] - off_hi[g0]) // 128)
                        return [r for r in (lo, hi) if r[1] > r[0]]

                    # e = alpha_src + alpha_dst (per block: ad varies); the
                    # local table tile holds alpha_dst at cols ZCOL:DCAT
                    for b in range(g0, g1):
                        adc = table_sb[:, b * ROW + ZCOL:b * ROW + DCAT]
                        for (c0, c1) in blk_ranges(b):
                            kk = int(c1 - c0)
                            nc.vector.tensor_add(
                                Gv[:, c0:c1, DCAT:DCAT + h],
                                Gv[:, c0:c1, d_hid:d_hid + h],
                                adc.rearrange("p (o j) -> p o j",
                                              o=1).broadcast_to([128, kk, h]))
                    # exp(leakyrelu(e)) == exp(max(e, 0.2e)) -- group-wide
                    ev = Gv[:, :, DCAT:DCAT + h]
                    nc.vector.scalar_tensor_tensor(
                        ev, ev, NEG_SLOPE, ev, op0=ALU.mult, op1=ALU.max)
                    # per lo/hi half (finer pipelining): 12-wide expanded
                    # weights (packed) for the message mul, the 8-wide copy
                    # into cols d_hid:ZCOL (softmax denominator, overwrites
                    # gathered alpha_src which the e-adds consumed), and the
                    # fully-packed bf16 message multiply (2x DVE mode)
                    SE = sexp.tile([128, GK_MAX * d_hid], TDT, tag="SE")
                    for (c0, c1) in ((0, KLg), (KLg, Kg)):
                        kk = c1 - c0
                        if kk == 0:
                            continue
                        SEr = SE[:, c0 * d_hid:c1 * d_hid].rearrange(
                            "p (k j d) -> p k j d", j=h, d=dh)
                        nc.scalar.activation(
                            SEr,
                            Gv[:, c0:c1, DCAT:DCAT + h].rearrange(
                                "p k (j o) -> p k j o",
                                o=1).broadcast_to([128, kk, h, dh]),
                            AF.Exp)
                        nc.vector.tensor_copy(
                            Gv[:, c0:c1, d_hid:d_hid + h].rearrange(
                                "p k (j o) -> p k j o", o=1),
                            SEr[:, :, :, 0:1])
                        nc.vector.tensor_mul(
                            Gv[:, c0:c1, 0:d_hid], Gv[:, c0:c1, 0:d_hid],
                            SE[:, c0 * d_hid:c1 * d_hid].rearrange(
                                "p (k f) -> p k f", f=d_hid))
                    # per-block accumulate [msg | s_exp] via identity matmul
                    # into one group-wide PSUM tile (one batched epilogue)
                    psGf = psB.tile([128, GBMAX * 128], mybir.dt.float32,
                                    name="psG", tag="psG")
                    psG = psGf[:, :gb * 128]
                    for j, b in enumerate(range(g0, g1)):
                        chunks = [c for (c0, c1) in blk_ranges(b)
                                  for c in range(c0, c1)]
                        assert chunks, (g0, g1, b)
                        for i, c in enumerate(chunks):
                            nc.tensor.matmul(
                                psG[:, j * 128:j * 128 + ZCOL],
                                idt_sb[:, :],
                                G[:, c * 128:c * 128 + ZCOL],
                                start=(i == 0), stop=(i == len(chunks) - 1))
                    post_group(g0, gb, psG)

            # ======= helper: output MLP + log_softmax (batched blocks) =====
            # Per block: transpose + matmul into a shared multi-block PSUM
            # tile.  Every OUT_B blocks one batched epilogue computes
            # out = t0 - ln(sum(exp(t0))) over [128, OUT_B*n_cls] at once
            # (the max-shift is skipped: logits are O(1)).
            OUT_B = 7
            ostate = {"po": None, "b0": 0, "cnt": 0}

            def out_flush():
                nb, po = ostate["cnt"], ostate["po"]
                if not nb:
                    return
                b0 = ostate["b0"]
                w = nb * n_cls
                # biased logits straight into out_sb; the -ln(z) shift is
                # applied once, globally, after phase D (keeps ACT on the Exp
                # table the whole phase)
                ov = out_sb[:, b0 * n_cls:b0 * n_cls + w]
                nc.vector.tensor_add(
                    ov.rearrange("p (b c) -> p b c", c=n_cls),
                    po[:, 0:w].rearrange("p (b c) -> p b c", c=n_cls),
                    bo_sb[:, :].rearrange("p (o c) -> p o c",
                                          o=1).broadcast_to([128, nb, n_cls]))
                ex = stage.tile([128, OUT_B * n_cls], mybir.dt.float32,
                                tag="ex")
                nc.scalar.activation(ex[:, 0:w], ov, AF.Exp)
                nc.vector.reduce_sum(
                    s_all[:, b0:b0 + nb],
                    ex[:, 0:w].rearrange("p (b c) -> p b c", c=n_cls),
                    axis=mybir.AxisListType.X)
                ostate["po"] = None
                ostate["cnt"] = 0

            def out_finish():
                ls = small.tile([128, nblk], mybir.dt.float32, tag="ls")
                nc.scalar.activation(ls[:], s_all[:, :], AF.Ln)
                nc.vector.tensor_sub(
                    out_sb[:, :].rearrange("p (b c) -> p b c", c=n_cls),
                    out_sb[:, :].rearrange("p (b c) -> p b c", c=n_cls),
                    ls[:, :].rearrange("p (b o) -> p b o",
                                       o=1).broadcast_to([128, nblk, n_cls]))

            def transpose_h(hv):
                """[128, d_hid] bf16 -> [d_hid, 128] bf16 SBUF (copy on ACT,
                which is off the DVE critical path)."""
                pst = psT.tile([d_hid, 128], TDT, tag="ptr")
                nc.tensor.transpose(pst[:], hv, idt_sb[:, :])
                hT = stage.tile([d_hid, 128], TDT, tag="hT")
                nc.scalar.activation(hT[:], pst[:], AF.Copy)
                return hT

            def out_block(b, hv):
                """hv: [128, d_hid] bf16 SBUF view of layer-2 activations."""
                hT = transpose_h(hv)
                if ostate["po"] is None:
                    po7 = psO.tile([128, OUT_B * n_cls], mybir.dt.float32,
                                   tag="po", name="po7")
                    ostate["po"] = po7
                    ostate["b0"] = b
                i = ostate["cnt"]
                nc.tensor.matmul(
                    ostate["po"][:, i * n_cls:(i + 1) * n_cls],
                    hT[:, :], Wo_sb[:, :], start=True, stop=True)
                ostate["cnt"] = i + 1
                if ostate["cnt"] == OUT_B:
                    out_flush()

            bailed = False

            # ================= phase A: table 0 =================
            for _rep in range(repeat):
              tfull0, tfull1 = tfulls[_rep]
              table_build(
                  lambda b: xT_sb[:, b * 128:(b + 1) * 128],
                  W0_sb, table0_sb, tfull0)

              if stop_after == "A":
                  bailed = True

              # ========== phase B: layer-0 edges + fused table-1 build =====
              # the layer-1 table pipeline (transpose/matmul/convert/ship)
              # for block b is emitted right in post0 so it streams through
              # phase B instead of bunching at the B->C boundary
              def norm_group(h_sb, bias_sb, b0, gb, psG):
                  """h[b0:b0+gb] = psG.msg / (psG.z + eps) + bias, batched."""
                  hv = h_sb[:, b0 * d_hid:(b0 + gb) * d_hid]
                  hv3 = hv.rearrange("p (g f) -> p g f", f=d_hid)
                  bb = bias_sb[:, :].rearrange(
                      "p (o f) -> p o f", o=1).broadcast_to([128, gb, d_hid])
                  if psG is None:
                      nc.vector.tensor_copy(hv3, bb)
                      return hv
                  psv = psG.rearrange("p (g z) -> p g z", z=128)
                  zf = small.tile([128, GBMAX * h], mybir.dt.float32, tag="z")
                  z = zf[:, :gb * h]
                  nc.vector.tensor_scalar_add(
                      z.rearrange("p (g j) -> p g j", j=h),
                      psv[:, :, d_hid:ZCOL], 1e-16)
                  izf = small.tile([128, GBMAX * h], mybir.dt.float32,
                                   tag="iz")
                  iz = izf[:, :gb * h]
                  nc.vector.reciprocal(iz, z)
                  izb = iz.rearrange("p (g j o) -> p g j o", o=1,
                                     j=h).broadcast_to([128, gb, h, dh])
                  nc.vector.tensor_mul(
                      hv.rearrange("p (g j d) -> p g j d", j=h, d=dh),
                      psv[:, :, 0:d_hid].rearrange("p g (j d) -> p g j d",
                                                   d=dh), izb)
                  nc.vector.tensor_add(hv3, hv3, bb)
                  return hv

              def post0(b0, gb, psG):
                  norm_group(h1_sb, b0_sb, b0, gb, psG)
                  for b in range(b0, b0 + gb):
                      hv = h1_sb[:, b * d_hid:(b + 1) * d_hid]
                      table_block(transpose_h(hv)[:, :], W1_sb, table1_sb, b)
                      if b + 1 in C_BOUNDS:
                          ci = C_BOUNDS.index(b + 1)
                          table_ship(table1_sb, tfull1, C_BOUNDS[ci - 1],
                                     b + 1)

              if not bailed:
                  edge_phase(tfull0, table0_sb, post0)
                  if not mock_cc:
                      nc.gpsimd.collective_compute(
                          "AllGather", mybir.AluOpType.bypass,
                          replica_groups=[list(range(cores))],
                          ins=[tslice[:, :]], outs=[tfull1[:, :]])
              if stop_after == "B":
                  bailed = True

              if stop_after == "C":
                  bailed = True

              # ======== phase D: layer-1 edges + fused output MLP ========
              # out_block emission lags post1 by a few blocks so the next
              # group's gather-releasing accumulate matmuls get PE priority
              pend = []

              def post1(b0, gb, psG):
                  hv = norm_group(h2_sb, b1_sb, b0, gb, psG)
                  nc.vector.tensor_scalar_max(hv, hv, 0.0)  # ReLU, in place
                  for b in range(b0, b0 + gb):
                      pend.append((b, h2_sb[:, b * d_hid:(b + 1) * d_hid]))
                  while len(pend) > 3:
                      out_block(*pend.pop(0))

              if not bailed:
                  edge_phase(tfull1, table1_sb, post1)
                  while pend:
                      out_block(*pend.pop(0))
                  out_flush()
                  out_finish()

            if bailed:
                nc.vector.memset(out_sb[:, :], 0.0)
            # one contiguous DMA: out row r = p*nblk + b  <=>  [p, (b c)]
            nc.sync.dma_start(
                out_d[:, :].rearrange("(p b) c -> p (b c)", p=128),
                out_sb[:, :])

    nc.compile()
    return nc


# ---------------------------------------------------------------- inputs
def make_in_maps(plan, inputs, f_in=F_IN, d_hid=D_HID, h=H, n_cls=N_CLS,
                 bf16=True):
    import ml_dtypes

    x = np.asarray(inputs["x"], dtype=np.float32)
    W0 = np.asarray(inputs["W0"], dtype=np.float32)
    W1 = np.asarray(inputs["W1"], dtype=np.float32)
    Wout = np.asarray(inputs["Wout"], dtype=np.float32)
    as0 = np.asarray(inputs["as0"], dtype=np.float32)
    ad0 = np.asarray(inputs["ad0"], dtype=np.float32)
    as1 = np.asarray(inputs["as1"], dtype=np.float32)
    ad1 = np.asarray(inputs["ad1"], dtype=np.float32)
    b0 = np.asarray(inputs["b0"], dtype=np.float32)
    b1 = np.asarray(inputs["b1"], dtype=np.float32)
    bout = np.asarray(inputs["bout"], dtype=np.float32)

    dh = d_hid // h
    npc, cores = plan["npc"], plan["cores"]
    old_of_slot = plan["old_of_slot"]

    def blockdiag(a):  # [h, dh] -> [d_hid, h]
        m = np.zeros((d_hid, h), dtype=np.float32)
        for j in range(h):
            m[j * dh:(j + 1) * dh, j] = a[j]
        return m

    W0cat = np.concatenate(
        [W0, W0 @ blockdiag(as0), W0 @ blockdiag(ad0)], axis=1)
    W1cat = np.concatenate(
        [W1, W1 @ blockdiag(as1), W1 @ blockdiag(ad1)], axis=1)

    tdt = ml_dtypes.bfloat16
    ident = np.eye(128, dtype=tdt)
    b0b = np.ascontiguousarray(np.broadcast_to(b0, (128, d_hid)))
    b1b = np.ascontiguousarray(np.broadcast_to(b1, (128, d_hid)))
    boutb = np.ascontiguousarray(np.broadcast_to(bout, (128, n_cls)))

    x_ext = np.zeros((npc * cores, f_in), dtype=np.float32)
    real = old_of_slot >= 0
    x_ext[real] = x[old_of_slot[real]]

    in_maps = []
    for c in range(cores):
        xs = x_ext[c * npc:(c + 1) * npc]
        m = dict(
            xT=np.ascontiguousarray(xs.T).astype(tdt),
            W0cat=W0cat.astype(tdt), W1cat=W1cat.astype(tdt),
            Wout=Wout.astype(tdt),
            b0b=b0b, b1b=b1b, boutb=boutb,
            identt=ident,
            idx_lo=np.ascontiguousarray(plan["idx_lo"][c])
            if plan["S_lo"] else np.zeros((128, 1), np.int16),
            idx_hi=np.ascontiguousarray(plan["idx_hi"][c])
            if plan["S_hi"] else np.zeros((128, 1), np.int16),
        )
        in_maps.append(m)
    return in_maps


def assemble_output(plan, results, n_cls=N_CLS):
    outs = np.concatenate([r["out"] for r in results], axis=0)
    return np.ascontiguousarray(outs[plan["row_of_old"]], dtype=np.float32)


# ---------------------------------------------------------------- entry
def kernel(**inputs):
    from concourse.bass_utils import run_bass_kernel_spmd

    edge_src = np.asarray(inputs["edge_src"]).astype(np.int64)
    edge_dst = np.asarray(inputs["edge_dst"]).astype(np.int64)

    bf16 = True
    key = (edge_src.tobytes(), edge_dst.tobytes(), bf16)
    kh = hash(key)
    if kh not in _CACHE:
        plan = make_plan(edge_src, edge_dst)
        nc = build_program(plan, bf16=bf16)
        _CACHE[kh] = (plan, nc)
    plan, nc = _CACHE[kh]

    in_maps = make_in_maps(plan, inputs, bf16=bf16)
    res = run_bass_kernel_spmd(nc, in_maps,
                               core_ids=list(range(plan["cores"])))
    return assemble_output(plan, res.results)
